# revision 24
# baseline (speedup 1.0000x reference)
"""Trainium2 Bass kernel for GQA attention with RoPE (dense_transformer).

Reference computation (per batch b):
    q = x @ wq  -> [T, 32, 64],  k = x @ wk -> [T, 8, 64], v = x @ wv
    rope(q), rope(k); scores = q k^T / 8; w = softmax(scores); out = (w v) @ wo

Sharding over 8 NeuronCores: 2 batch groups x 4-way head tensor parallel.
Core c: batch b=c//4, head group g=c%4 (q-heads 8g..8g+8, kv-heads 2g,2g+1).
Within a group of 4 cores the attention outputs (transposed, [512,T]) are
AllGather'd per 512-column t-chunk; each core then computes a 512-column
slice of out = attn @ wo.

v4 schedule — v3 merged pipeline plus:
  - Batched DMA dispatch: the sync engine serializes dma_start dispatch
    at ~600ns each, so the 16 per-chunk AllGather unloads become 1 wide
    dispatch, wo weights 2, x^T quarters 1-3 use 4 each, and the 4
    per-chunk output stores become 1 (the AG latency itself is a ~16-25us
    fixed cost regardless of payload, so splitting collectives does not
    help; batching the dispatches removes the sync-queue head-of-line
    blocking that stalled the PE mid-kernel).
  - Chunk 3 endgame is a per-tt chase: 4 column-sliced unloads of the
    last gather, each wo group starting as soon as its slice lands, with
    per-tt output stores and ACT/DVE alternating psum evacuation.
"""

import numpy as np
import ml_dtypes

import concourse.bass as bass
import concourse.mybir as mybir
import concourse.tile as tile
from concourse import bacc
from concourse.bass_utils import run_bass_kernel_spmd

BF16 = mybir.dt.bfloat16
F32 = mybir.dt.float32
I16 = mybir.dt.int16

T = 2048          # sequence length (also s dim)
C = 2048          # model dim
HD = 64           # head dim
DQ = 512          # q dims per core (8 heads)
DKV = 128         # kv dims per core (2 kv heads)
N_CORES = 8
THETA = 10000.0

EXP = mybir.ActivationFunctionType.Exp
COPY = mybir.ActivationFunctionType.Copy
MULT = mybir.AluOpType.mult
ADD = mybir.AluOpType.add

# Schraudolph exp producing bf16 BITS via one DVE tensor_scalar:
# bf16_bits(e^x) ~= int16(x * 128/ln2 + (127<<7) - 0.0579*128)
EXP_A = 128.0 / float(np.log(2.0))
EXP_B = 16256.0 - 0.0579 * 128.0
NPBF16 = ml_dtypes.bfloat16


def build_nc():
    nc = bacc.Bacc()

    xT_d = nc.declare_dram_parameter("xT", [C, T], BF16, isOutput=False)
    wkvq_d = nc.declare_dram_parameter("wkvq", [C, 2 * DKV + DQ], BF16, isOutput=False)
    wo_d = nc.declare_dram_parameter("wo", [C, DQ], BF16, isOutput=False)
    cosr_d = nc.declare_dram_parameter("cosr", [128, T], BF16, isOutput=False)
    sinr_d = nc.declare_dram_parameter("sinr", [128, T], BF16, isOutput=False)
    out_d = nc.declare_dram_parameter("out", [T, DQ], F32, isOutput=True)

    with tile.TileContext(nc) as tc:
        with (
            tc.tile_pool(name="persist", bufs=1) as pp,
            tc.tile_pool(name="dram", bufs=1, space="DRAM") as dp,
        ):
            # ---------- persistent SBUF ----------
            # roped Q^T tiles: qt[p] holds local heads (2p, 2p+1) on partitions
            # [0:64] / [64:128]; free dim = t
            qt = [pp.tile([128, T], BF16, tag=f"qt{i}", name=f"qt{i}") for i in range(4)]
            # duplicated roped K^T tiles: ktd[j] = [kv_j ; kv_j]
            ktd = [pp.tile([128, T], BF16, tag=f"ktd{i}", name=f"ktd{i}") for i in range(2)]
            # V augmented with a ones column, padded to 128 stationary cols
            # (full-width weights enable fast-weight-load):
            # per kv head, per s-tile [128, 128] = [v(64) | ones | zeros]
            vaug = [
                [pp.tile([128, 128], BF16, tag=f"va{j}_{s}", name=f"va{j}_{s}") for s in range(16)]
                for j in range(2)
            ]
            cosr = pp.tile([128, T], BF16, tag="cosr")
            sinr = pp.tile([128, T], BF16, tag="sinr")
            # wo weights, one wide tile; 512-col group j holds the wo rows for
            # gathered d-tile j (host pre-permutes rows into half-AG order)
            wo_w = pp.tile([128, 16 * 512], BF16, tag="wo_w", name="wo_w")
            # gathered attention: [128, 16*512]; col group m = d-tile m
            agw = pp.tile([128, 16 * 512], BF16, tag="agw", name="agw")

            for j in range(2):
                for s in range(16):
                    nc.gpsimd.memset(vaug[j][s][:, HD + 1:], 0.0)
                    nc.gpsimd.memset(vaug[j][s][:, HD:HD + 1], 1.0)
            # warm the ACT exp table set early so the ~2.7us ACT_TABLE_LOAD is
            # off the attention critical path
            warm = pp.tile([1, 8], F32, tag="warm")
            nc.gpsimd.memset(warm[:], 0.0)
            nc.scalar.activation(warm[:], warm[:], EXP)

            # ---------- DRAM bounce for AllGather (4 chunks of 512 t) ----------
            cc_in = [dp.tile([DQ, 512], BF16, tag=f"cci{i}", name=f"cci{i}") for i in range(4)]
            cc_out = [dp.tile([4 * DQ, 512], BF16, tag=f"cco{i}", name=f"cco{i}") for i in range(4)]
            # warmup collective: absorbs the DGE start delay (~11us) and the
            # initial cross-core sync skew so the first real AllGather is fast
            cw_in = dp.tile([128, 16], BF16, tag="cwi", name="cwi")
            cw_out = dp.tile([512, 16], BF16, tag="cwo", name="cwo")

            with (
                tc.tile_pool(name="pb", bufs=1) as pb,
                tc.tile_pool(name="pb_ps", bufs=1, space=bass.MemorySpace.PSUM) as bps,
            ):
                wkvq_sb = [
                    pb.tile([128, 2 * DKV + DQ], BF16, tag=f"wkvq{i}", name=f"wkvq{i}")
                    for i in range(16)
                ]
                wk_sb = [t[:, 0:DKV] for t in wkvq_sb]
                wv_sb = [t[:, DKV:2 * DKV] for t in wkvq_sb]
                wq_sb = [t[:, 2 * DKV:2 * DKV + DQ] for t in wkvq_sb]
                ktraw = pb.tile([128, T], BF16, tag="ktraw")

                # ---------- warmups ----------
                junk = pb.tile([128, 512], BF16, tag="junk")
                nc.gpsimd.memset(junk[:], 0.0)
                nc.sync.dma_start(out=cw_in[:], in_=junk[:, 0:16])
                nc.gpsimd.collective_compute(
                    "AllGather",
                    mybir.AluOpType.bypass,
                    replica_groups=[[0, 1, 2, 3], [4, 5, 6, 7]],
                    ins=[cw_in[:].opt()],
                    outs=[cw_out[:].opt()],
                )
                for _ in range(10):
                    jps = bps.tile([128, 1024], F32, tag="qk2", bufs=3, name="jps")
                    nc.tensor.matmul(jps[:, 0:512], junk[:, 0:128], junk[:], start=True, stop=True)

                # ---------- x^T tiles ----------
                # wide per-quarter tiles [128, 16*512]; col group kc holds
                # xT[kc*128:(kc+1)*128, q*512:(q+1)*512].  bufs=3: quarter q+3
                # reuses quarter q's slot once its last consumer retires.
                xtw = {}

                def xt(kc, q):
                    return xtw[q][:, kc * 512:(kc + 1) * 512]

                def emit_xtq_dma(q):
                    t = pb.tile([128, 16 * 512], BF16, tag="xtw", bufs=3, name=f"xtw_{q}")
                    xtw[q] = t
                    # 4 dispatches of 4 kc-tiles each
                    for g in range(4):
                        nc.sync.dma_start(
                            out=t[:, g * 2048:(g + 1) * 2048].rearrange(
                                "p (kc t) -> p kc t", kc=4
                            ),
                            in_=xT_d[g * 512:(g + 1) * 512,
                                     q * 512:(q + 1) * 512].rearrange(
                                "(kc p) t -> p kc t", p=128
                            ),
                        )

                # quarter 0 as 16 separate dispatches (all DMA rings in
                # parallel -- latency-critical), interleaved kc-major with the
                # weights so the K-proj chain's operands arrive consumer-order
                xtw[0] = pb.tile([128, 16 * 512], BF16, tag="xtw", bufs=3, name="xtw_0")
                for kc in range(16):
                    nc.sync.dma_start(
                        out=wkvq_sb[kc][:], in_=wkvq_d[kc * 128:(kc + 1) * 128, :]
                    )
                    nc.sync.dma_start(
                        out=xtw[0][:, kc * 512:(kc + 1) * 512],
                        in_=xT_d[kc * 128:(kc + 1) * 128, 0:512],
                    )
                emit_xtq_dma(1)
                nc.sync.dma_start(out=cosr[:], in_=cosr_d[:])
                nc.sync.dma_start(out=sinr[:], in_=sinr_d[:])
                # xtw q2/q3 and wo stream in during the prologue tail / chunk
                # 0 via deferred emission (the DMA dispatch queue is serial at
                # ~600ns per descriptor: consumer order = dispatch order)

                # ---------- projection building blocks ----------
                def emit_k_quarter(q):
                    ps = bps.tile([128, 1024], F32, tag="qk2", bufs=3, name="kps")
                    for kc in range(16):
                        nc.tensor.matmul(
                            ps[:, 0:512], wk_sb[kc], xt(kc, q),
                            start=(kc == 0), stop=(kc == 15),
                        )
                    nc.vector.tensor_copy(ktraw[:, q * 512:(q + 1) * 512], ps[:, 0:512])

                def emit_v_slice(s):
                    q, qo = s // 4, (s % 4) * 128
                    psv = bps.tile([128, 512], F32, tag="pv", bufs=2, name="vps")
                    for kc in range(16):
                        nc.tensor.matmul(
                            psv[:, 0:128], xt(kc, q)[:, qo:qo + 128], wv_sb[kc],
                            start=(kc == 0), stop=(kc == 15),
                        )
                    nc.vector.tensor_copy(vaug[0][s][:, 0:HD], psv[:, 0:HD])
                    nc.vector.tensor_copy(vaug[1][s][:, 0:HD], psv[:, HD:2 * HD])

                def emit_krope_half(half):
                    # K rope on a [128,1024] half -> duplicated halves of ktd
                    t0 = half * 1024
                    ktr = pb.tile([128, 1024], BF16, tag="ktr", bufs=1)
                    swp = pb.tile([128, 1024], BF16, tag="swpk", bufs=1)
                    for a, b in ((0, 32), (32, 0), (64, 96), (96, 64)):
                        nc.sync.dma_start(out=swp[a:a + 32, :], in_=ktraw[b:b + 32, t0:t0 + 1024])
                    t1k = pb.tile([128, 1024], BF16, tag="t1k", bufs=1)
                    t2k = pb.tile([128, 1024], BF16, tag="t2k", bufs=1)
                    nc.vector.tensor_mul(t1k[:], ktraw[:, t0:t0 + 1024], cosr[:, t0:t0 + 1024])
                    nc.vector.tensor_mul(t2k[:], swp[:], sinr[:, t0:t0 + 1024])
                    nc.vector.tensor_add(ktr[:], t1k[:], t2k[:])
                    nc.sync.dma_start(out=ktd[0][0:64, t0:t0 + 1024], in_=ktr[0:64, :])
                    nc.sync.dma_start(out=ktd[0][64:128, t0:t0 + 1024], in_=ktr[0:64, :])
                    nc.sync.dma_start(out=ktd[1][0:64, t0:t0 + 1024], in_=ktr[64:128, :])
                    nc.sync.dma_start(out=ktd[1][64:128, t0:t0 + 1024], in_=ktr[64:128, :])

                def emit_q_group(q, dq):
                    """Q proj psum tile (dq, quarter q); evac on ACT."""
                    ps = bps.tile([128, 1024], F32, tag="qk2", bufs=3, name="qps")
                    for kc in range(16):
                        nc.tensor.matmul(
                            ps[:, 0:512],
                            wkvq_sb[kc][:, 2 * DKV + dq * 128:2 * DKV + (dq + 1) * 128],
                            xt(kc, q),
                            start=(kc == 0), stop=(kc == 15),
                        )
                    qr = pb.tile([128, 512], BF16, tag="qraw", bufs=3, name="qr")
                    nc.scalar.activation(qr[:], ps[:, 0:512], COPY)
                    return qr

                def emit_qrope_quarter(dq, q, qr):
                    # qt[dq][:, q-quarter] = qr*cosr + swap32(qr)*sinr
                    t0 = q * 512
                    swp = pb.tile([128, 512], BF16, tag="swpq", bufs=2)
                    for a, b in ((0, 32), (32, 0), (64, 96), (96, 64)):
                        nc.sync.dma_start(out=swp[a:a + 32, :], in_=qr[b:b + 32, :])
                    t1 = pb.tile([128, 512], BF16, tag="t1q", bufs=2)
                    t2 = pb.tile([128, 512], BF16, tag="t2q", bufs=2)
                    nc.vector.tensor_mul(t1[:], qr[:], cosr[:, t0:t0 + 512])
                    nc.vector.tensor_mul(t2[:], swp[:], sinr[:, t0:t0 + 512])
                    nc.vector.tensor_add(qt[dq][:, t0:t0 + 512], t1[:], t2[:])

                # ---------- prologue ----------
                # Q quarter 0 goes early so its xtw slot frees for quarter 3,
                # letting K finish (and ktd fully write) before the trailing
                # V block -- the first attention QK then starts stall-free.
                emit_k_quarter(0)
                emit_k_quarter(1)
                for s in range(4):
                    emit_v_slice(s)
                emit_krope_half(0)
                for dq in range(4):
                    qr = emit_q_group(0, dq)
                    emit_qrope_quarter(dq, 0, qr)
                emit_xtq_dma(2)
                emit_xtq_dma(3)
                for s in range(4, 8):
                    emit_v_slice(s)
                emit_k_quarter(2)
                emit_k_quarter(3)
                emit_krope_half(1)
                for s in range(8, 16):
                    emit_v_slice(s)

                # ---------- attention pair blocks ----------
                def new_ctx(chunk, pair):
                    return {
                        "chunk": chunk, "pair": pair,
                        "ta": chunk * 512, "kv": pair // 2,
                        "pv_a": bps.tile([128, 512], F32, tag="pv", bufs=2, name="pv_a"),
                        "pv_b": bps.tile([128, 512], F32, tag="pv", bufs=2, name="pv_b"),
                        "qks": {}, "ess": {},
                    }

                def emit_qk(ctx, s):
                    qk = bps.tile([128, 1024], F32, tag="qk2", bufs=3)
                    ctx["qks"][s] = qk
                    kv, pair, ta = ctx["kv"], ctx["pair"], ctx["ta"]
                    # row-packed pair: head A on rows 0-63 -> tile (0,0),
                    # head B on rows 64-127 -> tile (64,0): concurrent MMs
                    nc.tensor.matmul(
                        qk[:, 0:512],
                        ktd[kv][0:64, s * 128:(s + 1) * 128],
                        qt[pair][0:64, ta:ta + 512],
                        start=True, stop=True,
                    )
                    nc.tensor.matmul(
                        qk[:, 512:1024],
                        ktd[kv][64:128, s * 128:(s + 1) * 128],
                        qt[pair][64:128, ta:ta + 512],
                        start=True, stop=True,
                    )

                def emit_exp(ctx, s):
                    # full-tile exp alternating engines per s (one dispatch
                    # bubble per 1024 cols); bufs=4 (even) so each pool slot
                    # is always rewritten by the SAME engine
                    qk = ctx["qks"][s]
                    es = pb.tile([128, 1024], BF16, tag="es", bufs=4)
                    ctx["ess"][s] = es
                    if s % 2 == 0:
                        nc.scalar.activation(es[:], qk[:], EXP)
                    else:
                        nc.vector.tensor_scalar(
                            es[:].bitcast(I16), qk[:],
                            EXP_A, EXP_B, MULT, ADD,
                        )

                def emit_pv(ctx, s):
                    es = ctx["ess"].pop(s)
                    ctx["qks"].pop(s)
                    kv = ctx["kv"]
                    nc.tensor.matmul(
                        ctx["pv_a"][:], vaug[kv][s][:], es[:, 0:512],
                        start=(s == 0), stop=(s == 15),
                        skip_group_check=True,
                    )
                    nc.tensor.matmul(
                        ctx["pv_b"][:], vaug[kv][s][:], es[:, 512:1024],
                        start=(s == 0), stop=(s == 15),
                        skip_group_check=True,
                    )

                def emit_prologue(ctx):
                    # 2-deep lookahead: QK runs two iterations ahead of PV so
                    # the exp latency stays off the PE critical path; both
                    # first exps precede the previous pair's tail in engine
                    # queue order
                    emit_qk(ctx, 0)
                    emit_qk(ctx, 1)
                    emit_exp(ctx, 0)
                    emit_exp(ctx, 1)

                def emit_body(ctx):
                    for s in range(16):
                        if s + 2 < 16:
                            emit_qk(ctx, s + 2)
                            emit_exp(ctx, s + 2)
                        emit_pv(ctx, s)

                def emit_tail(ctx, fast_chain=False):
                    chunk, pair = ctx["chunk"], ctx["pair"]
                    pv_a, pv_b = ctx["pv_a"], ctx["pv_b"]
                    # evacuate pv (frees the PSUM banks for the next pair:
                    # ACT takes head A, DVE takes head B, in parallel)
                    pvs_a = pb.tile([HD + 1, 512], F32, tag="pvsa", bufs=1)
                    pvs_b = pb.tile([HD + 1, 512], F32, tag="pvsb", bufs=1)
                    nc.scalar.activation(pvs_a[:], pv_a[0:HD + 1, :], COPY)
                    nc.vector.tensor_copy(pvs_b[:], pv_b[0:HD + 1, :])
                    # denominator -> broadcast -> normalize -> cc_in
                    # (den rows reach partition 0 via DMA for the broadcast)
                    denr = pb.tile([1, 1024], F32, tag="denr", bufs=2)
                    nc.scalar.activation(denr[0:1, 0:512], pv_a[64:65, :], COPY)
                    nc.scalar.activation(denr[0:1, 512:1024], pv_b[64:65, :], COPY)
                    denb = pb.tile([64, 1024], F32, tag="denb", bufs=1)
                    nc.gpsimd.partition_broadcast(denb[:], denr[0:1, :], channels=64)
                    tma = pb.tile([64, 512], BF16, tag="tma", bufs=2)
                    tmb = pb.tile([64, 512], BF16, tag="tmb", bufs=2)
                    rep = pb.tile([64, 1024], F32, tag="rep", bufs=1)
                    nc.vector.reciprocal_approx_fast(out=rep[:], in_=denb[:])
                    nc.vector.tensor_mul(tma[:], pvs_a[0:64, :], rep[:, 0:512])
                    nc.vector.tensor_mul(tmb[:], pvs_b[0:64, :], rep[:, 512:1024])
                    r0 = pair * 128
                    nc.sync.dma_start(out=cc_in[chunk][r0:r0 + 64, :], in_=tma[:])
                    nc.sync.dma_start(out=cc_in[chunk][r0 + 64:r0 + 128, :], in_=tmb[:])

                def do_ag_chunk(chunk):
                    nc.gpsimd.collective_compute(
                        "AllGather",
                        mybir.AluOpType.bypass,
                        replica_groups=[[0, 1, 2, 3], [4, 5, 6, 7]],
                        ins=[cc_in[chunk][:].opt()],
                        outs=[cc_out[chunk][:].opt()],
                    )

                def emit_ag_load(chunk):
                    # one wide dispatch: [2048, 512] DRAM -> [128, 16, 512] SBUF
                    # (bass sprays large single descriptors across DMA rings)
                    nc.sync.dma_start(
                        out=agw[:].rearrange("p (m t) -> p m t", m=16),
                        in_=cc_out[chunk][:].rearrange("(m p) t -> p m t", p=128),
                    )

                def emit_ag_load_tt(chunk, tt):
                    # per-tt column gather: [128, 16, 128] -- lets the final wo
                    # groups chase the AllGather instead of one bulk reload
                    nc.sync.dma_start(
                        out=agw[:].rearrange("p (m tt f) -> p m tt f", m=16, tt=4)[
                            :, :, tt:tt + 1, :],
                        in_=cc_out[chunk][:].rearrange("(m p) (tt f) -> p m tt f",
                                                       p=128, tt=4)[:, :, tt:tt + 1, :],
                    )

                def emit_wo_group(chunk, tt, osb_w, evac_dve=False):
                    """one [128, 512] psum tile of out[:, chunk cols]"""
                    pso = bps.tile([128, 1024], F32, tag="qk2", bufs=3, name="pso")
                    for m in range(16):
                        nc.tensor.matmul(
                            pso[:, 0:512],
                            agw[:, m * 512 + tt * 128:m * 512 + (tt + 1) * 128],
                            wo_w[:, m * 512:(m + 1) * 512],
                            start=(m == 0), stop=(m == 15),
                        )
                    if evac_dve:
                        nc.vector.tensor_copy(osb_w[:, tt * 512:(tt + 1) * 512], pso[:, 0:512])
                    else:
                        nc.scalar.activation(osb_w[:, tt * 512:(tt + 1) * 512], pso[:, 0:512], COPY)

                def emit_out_store(chunk, osb_w):
                    tb = chunk * 512
                    nc.sync.dma_start(
                        out=out_d[tb:tb + 512, :].rearrange("(tt p) c -> p tt c", p=128),
                        in_=osb_w[:].rearrange("p (tt c) -> p tt c", tt=4),
                    )

                def emit_out_store_tt(chunk, tt, osb_w):
                    tb = chunk * 512
                    nc.sync.dma_start(
                        out=out_d[tb + tt * 128:tb + (tt + 1) * 128, :],
                        in_=osb_w[:, tt * 512:(tt + 1) * 512],
                    )

                # ---------- schedule ----------
                # fillers[(chunk, pair)] -> emitted between the previous
                # pair's tail and this pair's body.
                # Q-proj fillers: the matmul group runs in pair p's slot;
                # its RoPE (DVE) runs in pair p+1's slot so it never delays
                # the exp stream right after the projection.
                qr_stash = {}

                def q_mm_filler(q, dq):
                    def f():
                        qr_stash[(q, dq)] = emit_q_group(q, dq)
                    return f

                def q_rope_filler(q, dq):
                    def f():
                        emit_qrope_quarter(dq, q, qr_stash.pop((q, dq)))
                    return f

                def wo_dma_filler():
                    def f():
                        for hh in range(2):
                            nc.sync.dma_start(
                                out=wo_w[:, hh * 4096:(hh + 1) * 4096].rearrange(
                                    "p (m c) -> p m c", m=8
                                ),
                                in_=wo_d[hh * 1024:(hh + 1) * 1024, :].rearrange(
                                    "(m p) c -> p m c", p=128
                                ),
                            )
                    return f

                fillers = {}
                fillers[(0, 1)] = [wo_dma_filler()]
                for c in (0, 1, 2):
                    q = c + 1
                    fillers.setdefault((c, 0), []).append(q_mm_filler(q, 0))
                    fillers.setdefault((c, 1), []).append(q_rope_filler(q, 0))
                    fillers[(c, 1)].append(q_mm_filler(q, 1))
                    fillers[(c, 2)] = [q_rope_filler(q, 1), q_mm_filler(q, 2)]
                    fillers[(c, 3)] = [q_rope_filler(q, 2), q_mm_filler(q, 3)]

                pending = {}
                prev = None
                deferred_store = None
                for chunk in range(4):
                    for pair in range(4):
                        ctx = pending.pop((chunk, pair), None)
                        if ctx is None:
                            ctx = new_ctx(chunk, pair)
                            emit_prologue(ctx)
                        if prev is not None:
                            emit_tail(prev)
                            prev = None
                        # the previous boundary's output store dispatches here,
                        # one pair later, so the sync queue never blocks on the
                        # wo psum evacuations still in flight
                        if pair == 1 and deferred_store is not None:
                            emit_out_store(*deferred_store)
                            deferred_store = None
                        # gathered-chunk unload: late enough that the AllGather
                        # is done (no DMA-queue head blocking), early enough
                        # that the boundary wo block never waits
                        if chunk >= 1 and pair == 3:
                            emit_ag_load(chunk - 1)
                        for f in fillers.get((chunk, pair), []):
                            f()
                        emit_body(ctx)
                        prev = ctx
                    if chunk < 3:
                        nctx = new_ctx(chunk + 1, 0)
                        emit_prologue(nctx)
                        pending[(chunk + 1, 0)] = nctx
                    emit_tail(prev, fast_chain=(chunk == 3))
                    prev = None
                    do_ag_chunk(chunk)
                    if chunk < 3:
                        emit_qrope_quarter(3, chunk + 1, qr_stash.pop((chunk + 1, 3)))
                    # chunk-boundary wo block: all four groups of chunk-1's
                    # AllGather (a full chunk of slack) run here as PE filler
                    # while this chunk's AllGather transfers
                    if chunk >= 1:
                        osb_w = pb.tile([128, 4 * 512], F32, tag="osb", bufs=1,
                                        name=f"osb{chunk - 1}")
                        for tt in range(4):
                            emit_wo_group(chunk - 1, tt, osb_w, evac_dve=(tt % 2 == 1))
                        if chunk < 3:
                            deferred_store = (chunk - 1, osb_w)
                        else:
                            emit_out_store(chunk - 1, osb_w)
                # final chunk: per-tt column unloads let each wo group start as
                # soon as its own slice of the gather has been pulled back in;
                # the per-tt WAR on the chunk-2 wo reads also releases early
                for tt in range(4):
                    emit_ag_load_tt(3, tt)
                osb_w = pb.tile([128, 4 * 512], F32, tag="osb", bufs=1, name="osb3")
                for tt in range(4):
                    emit_wo_group(3, tt, osb_w, evac_dve=(tt % 2 == 1))
                    emit_out_store_tt(3, tt, osb_w)

    return nc


# ---------------------------------------------------------------------------
# Host side
# ---------------------------------------------------------------------------

_CACHE = {}


def _rope_tables():
    i = np.arange(32)
    freqs = 1.0 / (THETA ** (2.0 * i / HD))          # [32]
    ang = np.arange(T, dtype=np.float64)[:, None] * freqs[None, :]  # [T, 32]
    cos = np.cos(ang)
    sin = np.sin(ang)
    p = np.arange(128)
    fi = p % 32
    sign = np.where(p % 64 < 32, -1.0, 1.0)
    cosr = cos[:, fi].T                               # [128, T]
    sinr = (sin[:, fi] * sign[None, :]).T             # [128, T]
    return cosr.astype(np.float32), sinr.astype(np.float32)


def _colperm(n_heads):
    """rotate-half permutation: per 64-col head block, evens then odds"""
    blk = np.concatenate([np.arange(0, HD, 2), np.arange(1, HD, 2)])
    return np.concatenate([h * HD + blk for h in range(n_heads)])


def _prep_in_maps(x, wq, wk, wv, wo):
    cosr, sinr = _rope_tables()
    qperm = _colperm(32)
    kperm = _colperm(8)
    wq_p = (wq.astype(np.float64) / 8.0)[:, qperm]    # fold 1/sqrt(hd)
    wk_p = wk[:, kperm]
    in_maps = []
    for c in range(N_CORES):
        b, g = divmod(c, 4)
        in_maps.append({
            "xT": np.ascontiguousarray(x[b].T).astype(NPBF16),
            "wkvq": np.concatenate(
                [wk_p[:, g * DKV:(g + 1) * DKV], wv[:, g * DKV:(g + 1) * DKV],
                 wq_p[:, g * DQ:(g + 1) * DQ]],
                axis=1,
            ).astype(NPBF16),
            "wo": wo[:, g * DQ:(g + 1) * DQ].astype(NPBF16),
            "cosr": cosr.astype(NPBF16),
            "sinr": sinr.astype(NPBF16),
        })
    return in_maps


def get_nc():
    if "nc" not in _CACHE:
        nc = build_nc()
        if not nc.is_finalized():
            nc.finalize()
        _CACHE["nc"] = nc
    return _CACHE["nc"]


def run_on_hw(in_maps, trace=False):
    nc = get_nc()
    return run_bass_kernel_spmd(nc, in_maps, core_ids=list(range(N_CORES)), trace=trace)


def _assemble(results):
    out = np.zeros((2, T, C), dtype=np.float32)
    for c in range(N_CORES):
        b, g = divmod(c, 4)
        out[b][:, g * DQ:(g + 1) * DQ] = np.asarray(results[c]["out"], dtype=np.float32)
    return out


def kernel(x, wq, wk, wv, wo):
    in_maps = _prep_in_maps(
        np.asarray(x, np.float32), np.asarray(wq, np.float32),
        np.asarray(wk, np.float32), np.asarray(wv, np.float32),
        np.asarray(wo, np.float32),
    )
    res = run_on_hw(in_maps, trace=False)
    return _assemble(res.results)


# revision 25
# speedup vs baseline: 1.0061x; 1.0061x over previous
"""Trainium2 Bass kernel for GQA attention with RoPE (dense_transformer).

Reference computation (per batch b):
    q = x @ wq  -> [T, 32, 64],  k = x @ wk -> [T, 8, 64], v = x @ wv
    rope(q), rope(k); scores = q k^T / 8; w = softmax(scores); out = (w v) @ wo

Sharding over 8 NeuronCores: 2 batch groups x 4-way head tensor parallel.
Core c: batch b=c//4, head group g=c%4 (q-heads 8g..8g+8, kv-heads 2g,2g+1).
Within a group of 4 cores the attention outputs (transposed, [512,T]) are
AllGather'd per 512-column t-chunk; each core then computes a 512-column
slice of out = attn @ wo.

v4 schedule — v3 merged pipeline plus:
  - Batched DMA dispatch: the sync engine serializes dma_start dispatch
    at ~600ns each, so the 16 per-chunk AllGather unloads become 1 wide
    dispatch, wo weights 2, x^T quarters 1-3 use 4 each, and the 4
    per-chunk output stores become 1 (the AG latency itself is a ~16-25us
    fixed cost regardless of payload, so splitting collectives does not
    help; batching the dispatches removes the sync-queue head-of-line
    blocking that stalled the PE mid-kernel).
  - Chunk 3 endgame is a per-tt chase: 4 column-sliced unloads of the
    last gather, each wo group starting as soon as its slice lands, with
    per-tt output stores and ACT/DVE alternating psum evacuation.
"""

import numpy as np
import ml_dtypes

import concourse.bass as bass
import concourse.mybir as mybir
import concourse.tile as tile
from concourse import bacc
from concourse.bass_utils import run_bass_kernel_spmd

BF16 = mybir.dt.bfloat16
F32 = mybir.dt.float32
I16 = mybir.dt.int16

T = 2048          # sequence length (also s dim)
C = 2048          # model dim
HD = 64           # head dim
DQ = 512          # q dims per core (8 heads)
DKV = 128         # kv dims per core (2 kv heads)
N_CORES = 8
THETA = 10000.0

EXP = mybir.ActivationFunctionType.Exp
COPY = mybir.ActivationFunctionType.Copy
MULT = mybir.AluOpType.mult
ADD = mybir.AluOpType.add

# Schraudolph exp producing bf16 BITS via one DVE tensor_scalar:
# bf16_bits(e^x) ~= int16(x * 128/ln2 + (127<<7) - 0.0579*128)
EXP_A = 128.0 / float(np.log(2.0))
EXP_B = 16256.0 - 0.0579 * 128.0
NPBF16 = ml_dtypes.bfloat16


def build_nc():
    nc = bacc.Bacc()

    xT_d = nc.declare_dram_parameter("xT", [C, T], BF16, isOutput=False)
    wkvq_d = nc.declare_dram_parameter("wkvq", [C, 2 * DKV + DQ], BF16, isOutput=False)
    wo_d = nc.declare_dram_parameter("wo", [C, DQ], BF16, isOutput=False)
    cosr_d = nc.declare_dram_parameter("cosr", [128, T], BF16, isOutput=False)
    sinr_d = nc.declare_dram_parameter("sinr", [128, T], BF16, isOutput=False)
    out_d = nc.declare_dram_parameter("out", [T, DQ], F32, isOutput=True)

    with tile.TileContext(nc) as tc:
        with (
            tc.tile_pool(name="persist", bufs=1) as pp,
            tc.tile_pool(name="dram", bufs=1, space="DRAM") as dp,
        ):
            # ---------- persistent SBUF ----------
            # roped Q^T tiles: qt[p] holds local heads (2p, 2p+1) on partitions
            # [0:64] / [64:128]; free dim = t
            qt = [pp.tile([128, T], BF16, tag=f"qt{i}", name=f"qt{i}") for i in range(4)]
            # duplicated roped K^T tiles: ktd[j] = [kv_j ; kv_j]
            ktd = [pp.tile([128, T], BF16, tag=f"ktd{i}", name=f"ktd{i}") for i in range(2)]
            # V augmented with a ones column, padded to 128 stationary cols
            # (full-width weights enable fast-weight-load):
            # per kv head, per s-tile [128, 128] = [v(64) | ones | zeros]
            vaug = [
                [pp.tile([128, 128], BF16, tag=f"va{j}_{s}", name=f"va{j}_{s}") for s in range(16)]
                for j in range(2)
            ]
            cosr = pp.tile([128, T], BF16, tag="cosr")
            sinr = pp.tile([128, T], BF16, tag="sinr")
            # wo weights, one wide tile; 512-col group j holds the wo rows for
            # gathered d-tile j (host pre-permutes rows into half-AG order)
            wo_w = pp.tile([128, 16 * 512], BF16, tag="wo_w", name="wo_w")
            # gathered attention: [128, 16*512]; col group m = d-tile m
            agw = pp.tile([128, 16 * 512], BF16, tag="agw", name="agw")

            for j in range(2):
                for s in range(16):
                    nc.gpsimd.memset(vaug[j][s][:, HD + 1:], 0.0)
                    nc.gpsimd.memset(vaug[j][s][:, HD:HD + 1], 1.0)
            # warm the ACT exp table set early so the ~2.7us ACT_TABLE_LOAD is
            # off the attention critical path
            warm = pp.tile([1, 8], F32, tag="warm")
            nc.gpsimd.memset(warm[:], 0.0)
            nc.scalar.activation(warm[:], warm[:], EXP)

            # ---------- DRAM bounce for AllGather (4 chunks of 512 t) ----------
            cc_in = [dp.tile([DQ, 512], BF16, tag=f"cci{i}", name=f"cci{i}") for i in range(4)]
            cc_out = [dp.tile([4 * DQ, 512], BF16, tag=f"cco{i}", name=f"cco{i}") for i in range(4)]
            # warmup collective: absorbs the DGE start delay (~11us) and the
            # initial cross-core sync skew so the first real AllGather is fast
            cw_in = dp.tile([128, 16], BF16, tag="cwi", name="cwi")
            cw_out = dp.tile([512, 16], BF16, tag="cwo", name="cwo")

            with (
                tc.tile_pool(name="pb", bufs=1) as pb,
                tc.tile_pool(name="pb_ps", bufs=1, space=bass.MemorySpace.PSUM) as bps,
            ):
                wkvq_sb = [
                    pb.tile([128, 2 * DKV + DQ], BF16, tag=f"wkvq{i}", name=f"wkvq{i}")
                    for i in range(16)
                ]
                wk_sb = [t[:, 0:DKV] for t in wkvq_sb]
                wv_sb = [t[:, DKV:2 * DKV] for t in wkvq_sb]
                wq_sb = [t[:, 2 * DKV:2 * DKV + DQ] for t in wkvq_sb]
                ktraw = pb.tile([128, T], BF16, tag="ktraw")

                # ---------- warmups ----------
                junk = pb.tile([128, 512], BF16, tag="junk")
                nc.gpsimd.memset(junk[:], 0.0)
                nc.sync.dma_start(out=cw_in[:], in_=junk[:, 0:16])
                nc.gpsimd.collective_compute(
                    "AllGather",
                    mybir.AluOpType.bypass,
                    replica_groups=[[0, 1, 2, 3], [4, 5, 6, 7]],
                    ins=[cw_in[:].opt()],
                    outs=[cw_out[:].opt()],
                )
                for _ in range(5):
                    jps = bps.tile([128, 1024], F32, tag="qk2", bufs=3, name="jps")
                    nc.tensor.matmul(jps[:, 0:512], junk[:, 0:128], junk[:], start=True, stop=True)

                # ---------- x^T tiles ----------
                # wide per-quarter tiles [128, 16*512]; col group kc holds
                # xT[kc*128:(kc+1)*128, q*512:(q+1)*512].  bufs=3: quarter q+3
                # reuses quarter q's slot once its last consumer retires.
                xtw = {}

                def xt(kc, q):
                    return xtw[q][:, kc * 512:(kc + 1) * 512]

                def emit_xtq_dma(q):
                    t = pb.tile([128, 16 * 512], BF16, tag="xtw", bufs=3, name=f"xtw_{q}")
                    xtw[q] = t
                    # 4 dispatches of 4 kc-tiles each
                    for g in range(4):
                        nc.sync.dma_start(
                            out=t[:, g * 2048:(g + 1) * 2048].rearrange(
                                "p (kc t) -> p kc t", kc=4
                            ),
                            in_=xT_d[g * 512:(g + 1) * 512,
                                     q * 512:(q + 1) * 512].rearrange(
                                "(kc p) t -> p kc t", p=128
                            ),
                        )

                # quarter 0 as 16 separate dispatches (all DMA rings in
                # parallel -- latency-critical), interleaved kc-major with the
                # weights so the K-proj chain's operands arrive consumer-order
                xtw[0] = pb.tile([128, 16 * 512], BF16, tag="xtw", bufs=3, name="xtw_0")
                for kc in range(16):
                    nc.sync.dma_start(
                        out=wkvq_sb[kc][:], in_=wkvq_d[kc * 128:(kc + 1) * 128, :]
                    )
                    nc.sync.dma_start(
                        out=xtw[0][:, kc * 512:(kc + 1) * 512],
                        in_=xT_d[kc * 128:(kc + 1) * 128, 0:512],
                    )
                emit_xtq_dma(1)
                nc.sync.dma_start(out=cosr[:], in_=cosr_d[:])
                nc.sync.dma_start(out=sinr[:], in_=sinr_d[:])
                # xtw q2/q3 and wo stream in during the prologue tail / chunk
                # 0 via deferred emission (the DMA dispatch queue is serial at
                # ~600ns per descriptor: consumer order = dispatch order)

                # ---------- projection building blocks ----------
                def emit_k_quarter(q):
                    ps = bps.tile([128, 1024], F32, tag="qk2", bufs=3, name="kps")
                    for kc in range(16):
                        nc.tensor.matmul(
                            ps[:, 0:512], wk_sb[kc], xt(kc, q),
                            start=(kc == 0), stop=(kc == 15),
                        )
                    nc.vector.tensor_copy(ktraw[:, q * 512:(q + 1) * 512], ps[:, 0:512])

                def emit_v_slice(s):
                    q, qo = s // 4, (s % 4) * 128
                    psv = bps.tile([128, 512], F32, tag="pv", bufs=2, name="vps")
                    for kc in range(16):
                        nc.tensor.matmul(
                            psv[:, 0:128], xt(kc, q)[:, qo:qo + 128], wv_sb[kc],
                            start=(kc == 0), stop=(kc == 15),
                        )
                    nc.vector.tensor_copy(vaug[0][s][:, 0:HD], psv[:, 0:HD])
                    nc.vector.tensor_copy(vaug[1][s][:, 0:HD], psv[:, HD:2 * HD])

                def emit_krope_half(half):
                    # K rope on a [128,1024] half -> duplicated halves of ktd
                    t0 = half * 1024
                    ktr = pb.tile([128, 1024], BF16, tag="ktr", bufs=1)
                    swp = pb.tile([128, 1024], BF16, tag="swpk", bufs=1)
                    for a, b in ((0, 32), (32, 0), (64, 96), (96, 64)):
                        nc.sync.dma_start(out=swp[a:a + 32, :], in_=ktraw[b:b + 32, t0:t0 + 1024])
                    t1k = pb.tile([128, 1024], BF16, tag="t1k", bufs=1)
                    t2k = pb.tile([128, 1024], BF16, tag="t2k", bufs=1)
                    nc.vector.tensor_mul(t1k[:], ktraw[:, t0:t0 + 1024], cosr[:, t0:t0 + 1024])
                    nc.vector.tensor_mul(t2k[:], swp[:], sinr[:, t0:t0 + 1024])
                    nc.vector.tensor_add(ktr[:], t1k[:], t2k[:])
                    nc.sync.dma_start(out=ktd[0][0:64, t0:t0 + 1024], in_=ktr[0:64, :])
                    nc.sync.dma_start(out=ktd[0][64:128, t0:t0 + 1024], in_=ktr[0:64, :])
                    nc.sync.dma_start(out=ktd[1][0:64, t0:t0 + 1024], in_=ktr[64:128, :])
                    nc.sync.dma_start(out=ktd[1][64:128, t0:t0 + 1024], in_=ktr[64:128, :])

                def emit_q_group(q, dq):
                    """Q proj psum tile (dq, quarter q); evac on ACT."""
                    ps = bps.tile([128, 1024], F32, tag="qk2", bufs=3, name="qps")
                    for kc in range(16):
                        nc.tensor.matmul(
                            ps[:, 0:512],
                            wkvq_sb[kc][:, 2 * DKV + dq * 128:2 * DKV + (dq + 1) * 128],
                            xt(kc, q),
                            start=(kc == 0), stop=(kc == 15),
                        )
                    qr = pb.tile([128, 512], BF16, tag="qraw", bufs=3, name="qr")
                    nc.scalar.activation(qr[:], ps[:, 0:512], COPY)
                    return qr

                def emit_qrope_quarter(dq, q, qr):
                    # qt[dq][:, q-quarter] = qr*cosr + swap32(qr)*sinr
                    t0 = q * 512
                    swp = pb.tile([128, 512], BF16, tag="swpq", bufs=2)
                    for a, b in ((0, 32), (32, 0), (64, 96), (96, 64)):
                        nc.sync.dma_start(out=swp[a:a + 32, :], in_=qr[b:b + 32, :])
                    t1 = pb.tile([128, 512], BF16, tag="t1q", bufs=2)
                    t2 = pb.tile([128, 512], BF16, tag="t2q", bufs=2)
                    nc.vector.tensor_mul(t1[:], qr[:], cosr[:, t0:t0 + 512])
                    nc.vector.tensor_mul(t2[:], swp[:], sinr[:, t0:t0 + 512])
                    nc.vector.tensor_add(qt[dq][:, t0:t0 + 512], t1[:], t2[:])

                # ---------- prologue ----------
                # Q quarter 0 goes early so its xtw slot frees for quarter 3,
                # letting K finish (and ktd fully write) before the trailing
                # V block -- the first attention QK then starts stall-free.
                emit_k_quarter(0)
                emit_k_quarter(1)
                for s in range(4):
                    emit_v_slice(s)
                emit_krope_half(0)
                for dq in range(4):
                    qr = emit_q_group(0, dq)
                    emit_qrope_quarter(dq, 0, qr)
                emit_xtq_dma(2)
                emit_xtq_dma(3)
                for s in range(4, 8):
                    emit_v_slice(s)
                emit_k_quarter(2)
                emit_k_quarter(3)
                emit_krope_half(1)
                for s in range(8, 16):
                    emit_v_slice(s)

                # ---------- attention pair blocks ----------
                def new_ctx(chunk, pair):
                    return {
                        "chunk": chunk, "pair": pair,
                        "ta": chunk * 512, "kv": pair // 2,
                        "pv_a": bps.tile([128, 512], F32, tag="pv", bufs=2, name="pv_a"),
                        "pv_b": bps.tile([128, 512], F32, tag="pv", bufs=2, name="pv_b"),
                        "qks": {}, "ess": {},
                    }

                def emit_qk(ctx, s):
                    qk = bps.tile([128, 1024], F32, tag="qk2", bufs=3)
                    ctx["qks"][s] = qk
                    kv, pair, ta = ctx["kv"], ctx["pair"], ctx["ta"]
                    # row-packed pair: head A on rows 0-63 -> tile (0,0),
                    # head B on rows 64-127 -> tile (64,0): concurrent MMs
                    nc.tensor.matmul(
                        qk[:, 0:512],
                        ktd[kv][0:64, s * 128:(s + 1) * 128],
                        qt[pair][0:64, ta:ta + 512],
                        start=True, stop=True,
                    )
                    nc.tensor.matmul(
                        qk[:, 512:1024],
                        ktd[kv][64:128, s * 128:(s + 1) * 128],
                        qt[pair][64:128, ta:ta + 512],
                        start=True, stop=True,
                    )

                def emit_exp(ctx, s):
                    # full-tile exp alternating engines per s (one dispatch
                    # bubble per 1024 cols); bufs=4 (even) so each pool slot
                    # is always rewritten by the SAME engine
                    qk = ctx["qks"][s]
                    es = pb.tile([128, 1024], BF16, tag="es", bufs=4)
                    ctx["ess"][s] = es
                    if s % 2 == 0:
                        nc.scalar.activation(es[:], qk[:], EXP)
                    else:
                        nc.vector.tensor_scalar(
                            es[:].bitcast(I16), qk[:],
                            EXP_A, EXP_B, MULT, ADD,
                        )

                def emit_pv(ctx, s):
                    es = ctx["ess"].pop(s)
                    ctx["qks"].pop(s)
                    kv = ctx["kv"]
                    nc.tensor.matmul(
                        ctx["pv_a"][:], vaug[kv][s][:], es[:, 0:512],
                        start=(s == 0), stop=(s == 15),
                        skip_group_check=True,
                    )
                    nc.tensor.matmul(
                        ctx["pv_b"][:], vaug[kv][s][:], es[:, 512:1024],
                        start=(s == 0), stop=(s == 15),
                        skip_group_check=True,
                    )

                def emit_prologue(ctx):
                    # 2-deep lookahead: QK runs two iterations ahead of PV so
                    # the exp latency stays off the PE critical path; both
                    # first exps precede the previous pair's tail in engine
                    # queue order
                    emit_qk(ctx, 0)
                    emit_qk(ctx, 1)
                    emit_exp(ctx, 0)
                    emit_exp(ctx, 1)

                def emit_body(ctx):
                    for s in range(16):
                        if s + 2 < 16:
                            emit_qk(ctx, s + 2)
                            emit_exp(ctx, s + 2)
                        emit_pv(ctx, s)

                def emit_tail(ctx, fast_chain=False):
                    chunk, pair = ctx["chunk"], ctx["pair"]
                    pv_a, pv_b = ctx["pv_a"], ctx["pv_b"]
                    # evacuate pv (frees the PSUM banks for the next pair:
                    # ACT takes head A, DVE takes head B, in parallel)
                    pvs_a = pb.tile([HD + 1, 512], F32, tag="pvsa", bufs=1)
                    pvs_b = pb.tile([HD + 1, 512], F32, tag="pvsb", bufs=1)
                    nc.scalar.activation(pvs_a[:], pv_a[0:HD + 1, :], COPY)
                    nc.vector.tensor_copy(pvs_b[:], pv_b[0:HD + 1, :])
                    # denominator -> broadcast -> normalize -> cc_in
                    # (den rows reach partition 0 via DMA for the broadcast)
                    denr = pb.tile([1, 1024], F32, tag="denr", bufs=2)
                    nc.scalar.activation(denr[0:1, 0:512], pv_a[64:65, :], COPY)
                    nc.scalar.activation(denr[0:1, 512:1024], pv_b[64:65, :], COPY)
                    denb = pb.tile([64, 1024], F32, tag="denb", bufs=1)
                    nc.gpsimd.partition_broadcast(denb[:], denr[0:1, :], channels=64)
                    tma = pb.tile([64, 512], BF16, tag="tma", bufs=2)
                    tmb = pb.tile([64, 512], BF16, tag="tmb", bufs=2)
                    rep = pb.tile([64, 1024], F32, tag="rep", bufs=1)
                    nc.vector.reciprocal_approx_fast(out=rep[:], in_=denb[:])
                    nc.vector.tensor_mul(tma[:], pvs_a[0:64, :], rep[:, 0:512])
                    nc.vector.tensor_mul(tmb[:], pvs_b[0:64, :], rep[:, 512:1024])
                    r0 = pair * 128
                    nc.sync.dma_start(out=cc_in[chunk][r0:r0 + 64, :], in_=tma[:])
                    nc.sync.dma_start(out=cc_in[chunk][r0 + 64:r0 + 128, :], in_=tmb[:])

                def do_ag_chunk(chunk):
                    nc.gpsimd.collective_compute(
                        "AllGather",
                        mybir.AluOpType.bypass,
                        replica_groups=[[0, 1, 2, 3], [4, 5, 6, 7]],
                        ins=[cc_in[chunk][:].opt()],
                        outs=[cc_out[chunk][:].opt()],
                    )

                def emit_ag_load(chunk):
                    # one wide dispatch: [2048, 512] DRAM -> [128, 16, 512] SBUF
                    # (bass sprays large single descriptors across DMA rings)
                    nc.sync.dma_start(
                        out=agw[:].rearrange("p (m t) -> p m t", m=16),
                        in_=cc_out[chunk][:].rearrange("(m p) t -> p m t", p=128),
                    )

                def emit_ag_load_tt(chunk, tt):
                    # per-tt column gather: [128, 16, 128] -- lets the final wo
                    # groups chase the AllGather instead of one bulk reload
                    nc.sync.dma_start(
                        out=agw[:].rearrange("p (m tt f) -> p m tt f", m=16, tt=4)[
                            :, :, tt:tt + 1, :],
                        in_=cc_out[chunk][:].rearrange("(m p) (tt f) -> p m tt f",
                                                       p=128, tt=4)[:, :, tt:tt + 1, :],
                    )

                def emit_wo_group(chunk, tt, osb_w, evac_dve=False):
                    """one [128, 512] psum tile of out[:, chunk cols]"""
                    pso = bps.tile([128, 1024], F32, tag="qk2", bufs=3, name="pso")
                    for m in range(16):
                        nc.tensor.matmul(
                            pso[:, 0:512],
                            agw[:, m * 512 + tt * 128:m * 512 + (tt + 1) * 128],
                            wo_w[:, m * 512:(m + 1) * 512],
                            start=(m == 0), stop=(m == 15),
                        )
                    if evac_dve:
                        nc.vector.tensor_copy(osb_w[:, tt * 512:(tt + 1) * 512], pso[:, 0:512])
                    else:
                        nc.scalar.activation(osb_w[:, tt * 512:(tt + 1) * 512], pso[:, 0:512], COPY)

                def emit_out_store(chunk, osb_w):
                    tb = chunk * 512
                    nc.sync.dma_start(
                        out=out_d[tb:tb + 512, :].rearrange("(tt p) c -> p tt c", p=128),
                        in_=osb_w[:].rearrange("p (tt c) -> p tt c", tt=4),
                    )

                def emit_out_store_tt(chunk, tt, osb_w):
                    tb = chunk * 512
                    nc.sync.dma_start(
                        out=out_d[tb + tt * 128:tb + (tt + 1) * 128, :],
                        in_=osb_w[:, tt * 512:(tt + 1) * 512],
                    )

                # ---------- schedule ----------
                # fillers[(chunk, pair)] -> emitted between the previous
                # pair's tail and this pair's body.
                # Q-proj fillers: the matmul group runs in pair p's slot;
                # its RoPE (DVE) runs in pair p+1's slot so it never delays
                # the exp stream right after the projection.
                qr_stash = {}

                def q_mm_filler(q, dq):
                    def f():
                        qr_stash[(q, dq)] = emit_q_group(q, dq)
                    return f

                def q_rope_filler(q, dq):
                    def f():
                        emit_qrope_quarter(dq, q, qr_stash.pop((q, dq)))
                    return f

                def wo_dma_filler():
                    def f():
                        for hh in range(2):
                            nc.sync.dma_start(
                                out=wo_w[:, hh * 4096:(hh + 1) * 4096].rearrange(
                                    "p (m c) -> p m c", m=8
                                ),
                                in_=wo_d[hh * 1024:(hh + 1) * 1024, :].rearrange(
                                    "(m p) c -> p m c", p=128
                                ),
                            )
                    return f

                fillers = {}
                fillers[(0, 1)] = [wo_dma_filler()]
                for c in (0, 1, 2):
                    q = c + 1
                    fillers.setdefault((c, 0), []).append(q_mm_filler(q, 0))
                    fillers.setdefault((c, 1), []).append(q_rope_filler(q, 0))
                    fillers[(c, 1)].append(q_mm_filler(q, 1))
                    fillers[(c, 2)] = [q_rope_filler(q, 1), q_mm_filler(q, 2)]
                    fillers[(c, 3)] = [q_rope_filler(q, 2), q_mm_filler(q, 3)]

                pending = {}
                prev = None
                deferred_store = None
                for chunk in range(4):
                    for pair in range(4):
                        ctx = pending.pop((chunk, pair), None)
                        if ctx is None:
                            ctx = new_ctx(chunk, pair)
                            emit_prologue(ctx)
                        if prev is not None:
                            emit_tail(prev)
                            prev = None
                        # the previous boundary's output store dispatches here,
                        # one pair later, so the sync queue never blocks on the
                        # wo psum evacuations still in flight
                        if pair == 1 and deferred_store is not None:
                            emit_out_store(*deferred_store)
                            deferred_store = None
                        # gathered-chunk unload: late enough that the AllGather
                        # is done (no DMA-queue head blocking), early enough
                        # that the boundary wo block never waits
                        if chunk >= 1 and pair == 3:
                            emit_ag_load(chunk - 1)
                        for f in fillers.get((chunk, pair), []):
                            f()
                        emit_body(ctx)
                        prev = ctx
                    if chunk < 3:
                        nctx = new_ctx(chunk + 1, 0)
                        emit_prologue(nctx)
                        pending[(chunk + 1, 0)] = nctx
                    emit_tail(prev, fast_chain=(chunk == 3))
                    prev = None
                    do_ag_chunk(chunk)
                    if chunk < 3:
                        emit_qrope_quarter(3, chunk + 1, qr_stash.pop((chunk + 1, 3)))
                    # chunk-boundary wo block: all four groups of chunk-1's
                    # AllGather (a full chunk of slack) run here as PE filler
                    # while this chunk's AllGather transfers
                    if chunk >= 1:
                        osb_w = pb.tile([128, 4 * 512], F32, tag="osb", bufs=1,
                                        name=f"osb{chunk - 1}")
                        for tt in range(4):
                            emit_wo_group(chunk - 1, tt, osb_w, evac_dve=(tt % 2 == 1))
                        if chunk < 3:
                            deferred_store = (chunk - 1, osb_w)
                        else:
                            emit_out_store(chunk - 1, osb_w)
                # final chunk: per-tt column unloads let each wo group start as
                # soon as its own slice of the gather has been pulled back in;
                # the per-tt WAR on the chunk-2 wo reads also releases early
                for tt in range(4):
                    emit_ag_load_tt(3, tt)
                osb_w = pb.tile([128, 4 * 512], F32, tag="osb", bufs=1, name="osb3")
                for tt in range(4):
                    emit_wo_group(3, tt, osb_w, evac_dve=(tt % 2 == 1))
                    emit_out_store_tt(3, tt, osb_w)

    return nc


# ---------------------------------------------------------------------------
# Host side
# ---------------------------------------------------------------------------

_CACHE = {}


def _rope_tables():
    i = np.arange(32)
    freqs = 1.0 / (THETA ** (2.0 * i / HD))          # [32]
    ang = np.arange(T, dtype=np.float64)[:, None] * freqs[None, :]  # [T, 32]
    cos = np.cos(ang)
    sin = np.sin(ang)
    p = np.arange(128)
    fi = p % 32
    sign = np.where(p % 64 < 32, -1.0, 1.0)
    cosr = cos[:, fi].T                               # [128, T]
    sinr = (sin[:, fi] * sign[None, :]).T             # [128, T]
    return cosr.astype(np.float32), sinr.astype(np.float32)


def _colperm(n_heads):
    """rotate-half permutation: per 64-col head block, evens then odds"""
    blk = np.concatenate([np.arange(0, HD, 2), np.arange(1, HD, 2)])
    return np.concatenate([h * HD + blk for h in range(n_heads)])


def _prep_in_maps(x, wq, wk, wv, wo):
    cosr, sinr = _rope_tables()
    qperm = _colperm(32)
    kperm = _colperm(8)
    wq_p = (wq.astype(np.float64) / 8.0)[:, qperm]    # fold 1/sqrt(hd)
    wk_p = wk[:, kperm]
    in_maps = []
    for c in range(N_CORES):
        b, g = divmod(c, 4)
        in_maps.append({
            "xT": np.ascontiguousarray(x[b].T).astype(NPBF16),
            "wkvq": np.concatenate(
                [wk_p[:, g * DKV:(g + 1) * DKV], wv[:, g * DKV:(g + 1) * DKV],
                 wq_p[:, g * DQ:(g + 1) * DQ]],
                axis=1,
            ).astype(NPBF16),
            "wo": wo[:, g * DQ:(g + 1) * DQ].astype(NPBF16),
            "cosr": cosr.astype(NPBF16),
            "sinr": sinr.astype(NPBF16),
        })
    return in_maps


def get_nc():
    if "nc" not in _CACHE:
        nc = build_nc()
        if not nc.is_finalized():
            nc.finalize()
        _CACHE["nc"] = nc
    return _CACHE["nc"]


def run_on_hw(in_maps, trace=False):
    nc = get_nc()
    return run_bass_kernel_spmd(nc, in_maps, core_ids=list(range(N_CORES)), trace=trace)


def _assemble(results):
    out = np.zeros((2, T, C), dtype=np.float32)
    for c in range(N_CORES):
        b, g = divmod(c, 4)
        out[b][:, g * DQ:(g + 1) * DQ] = np.asarray(results[c]["out"], dtype=np.float32)
    return out


def kernel(x, wq, wk, wv, wo):
    in_maps = _prep_in_maps(
        np.asarray(x, np.float32), np.asarray(wq, np.float32),
        np.asarray(wk, np.float32), np.asarray(wv, np.float32),
        np.asarray(wo, np.float32),
    )
    res = run_on_hw(in_maps, trace=False)
    return _assemble(res.results)


# revision 26
# speedup vs baseline: 1.0194x; 1.0132x over previous
"""Trainium2 Bass kernel for GQA attention with RoPE (dense_transformer).

Reference computation (per batch b):
    q = x @ wq  -> [T, 32, 64],  k = x @ wk -> [T, 8, 64], v = x @ wv
    rope(q), rope(k); scores = q k^T / 8; w = softmax(scores); out = (w v) @ wo

Sharding over 8 NeuronCores: 2 batch groups x 4-way head tensor parallel.
Core c: batch b=c//4, head group g=c%4 (q-heads 8g..8g+8, kv-heads 2g,2g+1).
Within a group of 4 cores the attention outputs (transposed, [512,T]) are
AllGather'd per 512-column t-chunk; each core then computes a 512-column
slice of out = attn @ wo.

v4 schedule — v3 merged pipeline plus:
  - Batched DMA dispatch: the sync engine serializes dma_start dispatch
    at ~600ns each, so the 16 per-chunk AllGather unloads become 1 wide
    dispatch, wo weights 2, x^T quarters 1-3 use 4 each, and the 4
    per-chunk output stores become 1 (the AG latency itself is a ~16-25us
    fixed cost regardless of payload, so splitting collectives does not
    help; batching the dispatches removes the sync-queue head-of-line
    blocking that stalled the PE mid-kernel).
  - Chunk 3 endgame is a per-tt chase: 4 column-sliced unloads of the
    last gather, each wo group starting as soon as its slice lands, with
    per-tt output stores and ACT/DVE alternating psum evacuation.
"""

import numpy as np
import ml_dtypes

import concourse.bass as bass
import concourse.mybir as mybir
import concourse.tile as tile
from concourse import bacc
from concourse.bass_utils import run_bass_kernel_spmd

BF16 = mybir.dt.bfloat16
F32 = mybir.dt.float32
I16 = mybir.dt.int16

T = 2048          # sequence length (also s dim)
C = 2048          # model dim
HD = 64           # head dim
DQ = 512          # q dims per core (8 heads)
DKV = 128         # kv dims per core (2 kv heads)
N_CORES = 8
THETA = 10000.0

EXP = mybir.ActivationFunctionType.Exp
COPY = mybir.ActivationFunctionType.Copy
MULT = mybir.AluOpType.mult
ADD = mybir.AluOpType.add

# Schraudolph exp producing bf16 BITS via one DVE tensor_scalar:
# bf16_bits(e^x) ~= int16(x * 128/ln2 + (127<<7) - 0.0579*128)
EXP_A = 128.0 / float(np.log(2.0))
EXP_B = 16256.0 - 0.0579 * 128.0
NPBF16 = ml_dtypes.bfloat16


def build_nc():
    nc = bacc.Bacc()

    xT_d = nc.declare_dram_parameter("xT", [C, T], BF16, isOutput=False)
    wkvq_d = nc.declare_dram_parameter("wkvq", [C, 2 * DKV + DQ], BF16, isOutput=False)
    wo_d = nc.declare_dram_parameter("wo", [C, DQ], BF16, isOutput=False)
    cosr_d = nc.declare_dram_parameter("cosr", [128, T], BF16, isOutput=False)
    sinr_d = nc.declare_dram_parameter("sinr", [128, T], BF16, isOutput=False)
    out_d = nc.declare_dram_parameter("out", [T, DQ], F32, isOutput=True)

    with tile.TileContext(nc) as tc:
        with (
            tc.tile_pool(name="persist", bufs=1) as pp,
            tc.tile_pool(name="dram", bufs=1, space="DRAM") as dp,
        ):
            # ---------- persistent SBUF ----------
            # roped Q^T tiles: qt[p] holds local heads (2p, 2p+1) on partitions
            # [0:64] / [64:128]; free dim = t
            qt = [pp.tile([128, T], BF16, tag=f"qt{i}", name=f"qt{i}") for i in range(4)]
            # duplicated roped K^T tiles: ktd[j] = [kv_j ; kv_j]
            ktd = [pp.tile([128, T], BF16, tag=f"ktd{i}", name=f"ktd{i}") for i in range(2)]
            # V augmented with a ones column, padded to 128 stationary cols
            # (full-width weights enable fast-weight-load):
            # per kv head, per s-tile [128, 128] = [v(64) | ones | zeros]
            vaug = [
                [pp.tile([128, 128], BF16, tag=f"va{j}_{s}", name=f"va{j}_{s}") for s in range(16)]
                for j in range(2)
            ]
            cosr = pp.tile([128, T], BF16, tag="cosr")
            sinr = pp.tile([128, T], BF16, tag="sinr")
            # wo weights, one wide tile; 512-col group j holds the wo rows for
            # gathered d-tile j (host pre-permutes rows into half-AG order)
            wo_w = pp.tile([128, 16 * 512], BF16, tag="wo_w", name="wo_w")
            # gathered attention: [128, 16*512]; col group m = d-tile m
            agw = pp.tile([128, 16 * 512], BF16, tag="agw", name="agw")

            for j in range(2):
                for s in range(16):
                    nc.gpsimd.memset(vaug[j][s][:, HD + 1:], 0.0)
                    nc.gpsimd.memset(vaug[j][s][:, HD:HD + 1], 1.0)
            # warm the ACT exp table set early so the ~2.7us ACT_TABLE_LOAD is
            # off the attention critical path
            warm = pp.tile([1, 8], F32, tag="warm")
            nc.gpsimd.memset(warm[:], 0.0)
            nc.scalar.activation(warm[:], warm[:], EXP)

            # ---------- DRAM bounce for AllGather (4 chunks of 512 t) ----------
            cc_in = [dp.tile([DQ, 512], BF16, tag=f"cci{i}", name=f"cci{i}") for i in range(4)]
            cc_out = [dp.tile([4 * DQ, 512], BF16, tag=f"cco{i}", name=f"cco{i}") for i in range(4)]
            # warmup collective: absorbs the DGE start delay (~11us) and the
            # initial cross-core sync skew so the first real AllGather is fast
            cw_in = dp.tile([128, 16], BF16, tag="cwi", name="cwi")
            cw_out = dp.tile([512, 16], BF16, tag="cwo", name="cwo")

            with (
                tc.tile_pool(name="pb", bufs=1) as pb,
                tc.tile_pool(name="pb_ps", bufs=1, space=bass.MemorySpace.PSUM) as bps,
            ):
                wkvq_sb = [
                    pb.tile([128, 2 * DKV + DQ], BF16, tag=f"wkvq{i}", name=f"wkvq{i}")
                    for i in range(16)
                ]
                wk_sb = [t[:, 0:DKV] for t in wkvq_sb]
                wv_sb = [t[:, DKV:2 * DKV] for t in wkvq_sb]
                wq_sb = [t[:, 2 * DKV:2 * DKV + DQ] for t in wkvq_sb]
                ktraw = pb.tile([128, T], BF16, tag="ktraw")

                # ---------- warmups ----------
                junk = pb.tile([128, 512], BF16, tag="junk")
                nc.gpsimd.memset(junk[:], 0.0)
                nc.sync.dma_start(out=cw_in[:], in_=junk[:, 0:16])
                nc.gpsimd.collective_compute(
                    "AllGather",
                    mybir.AluOpType.bypass,
                    replica_groups=[[0, 1, 2, 3], [4, 5, 6, 7]],
                    ins=[cw_in[:].opt()],
                    outs=[cw_out[:].opt()],
                )
                for _ in range(5):
                    jps = bps.tile([128, 1024], F32, tag="qk2", bufs=3, name="jps")
                    nc.tensor.matmul(jps[:, 0:512], junk[:, 0:128], junk[:], start=True, stop=True)

                # ---------- x^T tiles ----------
                # wide per-quarter tiles [128, 16*512]; col group kc holds
                # xT[kc*128:(kc+1)*128, q*512:(q+1)*512].  bufs=3: quarter q+3
                # reuses quarter q's slot once its last consumer retires.
                xtw = {}

                def xt(kc, q):
                    return xtw[q][:, kc * 512:(kc + 1) * 512]

                def emit_xtq_dma(q):
                    t = pb.tile([128, 16 * 512], BF16, tag="xtw", bufs=3, name=f"xtw_{q}")
                    xtw[q] = t
                    # 4 dispatches of 4 kc-tiles each
                    for g in range(4):
                        nc.sync.dma_start(
                            out=t[:, g * 2048:(g + 1) * 2048].rearrange(
                                "p (kc t) -> p kc t", kc=4
                            ),
                            in_=xT_d[g * 512:(g + 1) * 512,
                                     q * 512:(q + 1) * 512].rearrange(
                                "(kc p) t -> p kc t", p=128
                            ),
                        )

                # quarter 0 as 16 separate dispatches (all DMA rings in
                # parallel -- latency-critical), interleaved kc-major with the
                # weights so the K-proj chain's operands arrive consumer-order
                xtw[0] = pb.tile([128, 16 * 512], BF16, tag="xtw", bufs=3, name="xtw_0")
                for kc in range(16):
                    nc.sync.dma_start(
                        out=wkvq_sb[kc][:], in_=wkvq_d[kc * 128:(kc + 1) * 128, :]
                    )
                    nc.sync.dma_start(
                        out=xtw[0][:, kc * 512:(kc + 1) * 512],
                        in_=xT_d[kc * 128:(kc + 1) * 128, 0:512],
                    )
                emit_xtq_dma(1)
                nc.sync.dma_start(out=cosr[:], in_=cosr_d[:])
                nc.sync.dma_start(out=sinr[:], in_=sinr_d[:])
                # xtw q2/q3 and wo stream in during the prologue tail / chunk
                # 0 via deferred emission (the DMA dispatch queue is serial at
                # ~600ns per descriptor: consumer order = dispatch order)

                # ---------- projection building blocks ----------
                def emit_k_quarter(q):
                    ps = bps.tile([128, 1024], F32, tag="qk2", bufs=3, name="kps")
                    for kc in range(16):
                        nc.tensor.matmul(
                            ps[:, 0:512], wk_sb[kc], xt(kc, q),
                            start=(kc == 0), stop=(kc == 15),
                        )
                    nc.vector.tensor_copy(ktraw[:, q * 512:(q + 1) * 512], ps[:, 0:512])

                def emit_v_slice(s):
                    q, qo = s // 4, (s % 4) * 128
                    psv = bps.tile([128, 512], F32, tag="pv", bufs=2, name="vps")
                    for kc in range(16):
                        nc.tensor.matmul(
                            psv[:, 0:128], xt(kc, q)[:, qo:qo + 128], wv_sb[kc],
                            start=(kc == 0), stop=(kc == 15),
                        )
                    nc.vector.tensor_copy(vaug[0][s][:, 0:HD], psv[:, 0:HD])
                    nc.vector.tensor_copy(vaug[1][s][:, 0:HD], psv[:, HD:2 * HD])

                def emit_krope_half(half):
                    # K rope on a [128,1024] half -> duplicated halves of ktd
                    t0 = half * 1024
                    ktr = pb.tile([128, 1024], BF16, tag="ktr", bufs=1)
                    swp = pb.tile([128, 1024], BF16, tag="swpk", bufs=1)
                    for a, b in ((0, 32), (32, 0), (64, 96), (96, 64)):
                        nc.sync.dma_start(out=swp[a:a + 32, :], in_=ktraw[b:b + 32, t0:t0 + 1024])
                    t1k = pb.tile([128, 1024], BF16, tag="t1k", bufs=1)
                    t2k = pb.tile([128, 1024], BF16, tag="t2k", bufs=1)
                    nc.vector.tensor_mul(t1k[:], ktraw[:, t0:t0 + 1024], cosr[:, t0:t0 + 1024])
                    nc.vector.tensor_mul(t2k[:], swp[:], sinr[:, t0:t0 + 1024])
                    nc.vector.tensor_add(ktr[:], t1k[:], t2k[:])
                    nc.sync.dma_start(out=ktd[0][0:64, t0:t0 + 1024], in_=ktr[0:64, :])
                    nc.sync.dma_start(out=ktd[0][64:128, t0:t0 + 1024], in_=ktr[0:64, :])
                    nc.sync.dma_start(out=ktd[1][0:64, t0:t0 + 1024], in_=ktr[64:128, :])
                    nc.sync.dma_start(out=ktd[1][64:128, t0:t0 + 1024], in_=ktr[64:128, :])

                def emit_q_group(q, dq):
                    """Q proj psum tile (dq, quarter q); evac on ACT."""
                    ps = bps.tile([128, 1024], F32, tag="qk2", bufs=3, name="qps")
                    for kc in range(16):
                        nc.tensor.matmul(
                            ps[:, 0:512],
                            wkvq_sb[kc][:, 2 * DKV + dq * 128:2 * DKV + (dq + 1) * 128],
                            xt(kc, q),
                            start=(kc == 0), stop=(kc == 15),
                        )
                    qr = pb.tile([128, 512], BF16, tag="qraw", bufs=3, name="qr")
                    nc.scalar.activation(qr[:], ps[:, 0:512], COPY)
                    return qr

                def emit_qrope_quarter(dq, q, qr):
                    # qt[dq][:, q-quarter] = qr*cosr + swap32(qr)*sinr
                    t0 = q * 512
                    swp = pb.tile([128, 512], BF16, tag="swpq", bufs=2)
                    for a, b in ((0, 32), (32, 0), (64, 96), (96, 64)):
                        nc.sync.dma_start(out=swp[a:a + 32, :], in_=qr[b:b + 32, :])
                    t1 = pb.tile([128, 512], BF16, tag="t1q", bufs=2)
                    t2 = pb.tile([128, 512], BF16, tag="t2q", bufs=2)
                    nc.vector.tensor_mul(t1[:], qr[:], cosr[:, t0:t0 + 512])
                    nc.vector.tensor_mul(t2[:], swp[:], sinr[:, t0:t0 + 512])
                    nc.vector.tensor_add(qt[dq][:, t0:t0 + 512], t1[:], t2[:])

                # ---------- prologue ----------
                # Q quarter 0 goes early so its xtw slot frees for quarter 3,
                # letting K finish (and ktd fully write) before the trailing
                # V block -- the first attention QK then starts stall-free.
                emit_k_quarter(0)
                emit_k_quarter(1)
                for s in range(4):
                    emit_v_slice(s)
                emit_krope_half(0)
                for dq in range(4):
                    qr = emit_q_group(0, dq)
                    emit_qrope_quarter(dq, 0, qr)
                emit_xtq_dma(2)
                emit_xtq_dma(3)
                for s in range(4, 8):
                    emit_v_slice(s)
                emit_k_quarter(2)
                emit_k_quarter(3)
                emit_krope_half(1)
                for s in range(8, 16):
                    emit_v_slice(s)

                # ---------- attention pair blocks ----------
                def new_ctx(chunk, pair):
                    return {
                        "chunk": chunk, "pair": pair,
                        "ta": chunk * 512, "kv": pair // 2,
                        "pv_a": bps.tile([128, 512], F32, tag="pv", bufs=2, name="pv_a"),
                        "pv_b": bps.tile([128, 512], F32, tag="pv", bufs=2, name="pv_b"),
                        "qks": {}, "ess": {},
                    }

                def emit_qk(ctx, s):
                    qk = bps.tile([128, 1024], F32, tag="qk2", bufs=3)
                    ctx["qks"][s] = qk
                    kv, pair, ta = ctx["kv"], ctx["pair"], ctx["ta"]
                    # row-packed pair: head A on rows 0-63 -> tile (0,0),
                    # head B on rows 64-127 -> tile (64,0): concurrent MMs
                    nc.tensor.matmul(
                        qk[:, 0:512],
                        ktd[kv][0:64, s * 128:(s + 1) * 128],
                        qt[pair][0:64, ta:ta + 512],
                        start=True, stop=True,
                    )
                    nc.tensor.matmul(
                        qk[:, 512:1024],
                        ktd[kv][64:128, s * 128:(s + 1) * 128],
                        qt[pair][64:128, ta:ta + 512],
                        start=True, stop=True,
                    )

                def emit_exp(ctx, s):
                    # full-tile exp alternating engines per s (one dispatch
                    # bubble per 1024 cols); bufs=4 (even) so each pool slot
                    # is always rewritten by the SAME engine
                    qk = ctx["qks"][s]
                    es = pb.tile([128, 1024], BF16, tag="es", bufs=4)
                    ctx["ess"][s] = es
                    if s % 2 == 0:
                        nc.scalar.activation(es[:], qk[:], EXP)
                    else:
                        nc.vector.tensor_scalar(
                            es[:].bitcast(I16), qk[:],
                            EXP_A, EXP_B, MULT, ADD,
                        )

                def emit_pv(ctx, s):
                    es = ctx["ess"].pop(s)
                    ctx["qks"].pop(s)
                    kv = ctx["kv"]
                    nc.tensor.matmul(
                        ctx["pv_a"][:], vaug[kv][s][:], es[:, 0:512],
                        start=(s == 0), stop=(s == 15),
                        skip_group_check=True,
                    )
                    nc.tensor.matmul(
                        ctx["pv_b"][:], vaug[kv][s][:], es[:, 512:1024],
                        start=(s == 0), stop=(s == 15),
                        skip_group_check=True,
                    )

                def emit_prologue(ctx):
                    # 2-deep lookahead: QK runs two iterations ahead of PV so
                    # the exp latency stays off the PE critical path; both
                    # first exps precede the previous pair's tail in engine
                    # queue order
                    emit_qk(ctx, 0)
                    emit_qk(ctx, 1)
                    emit_exp(ctx, 0)
                    emit_exp(ctx, 1)

                def emit_body(ctx):
                    for s in range(16):
                        if s + 2 < 16:
                            emit_qk(ctx, s + 2)
                            emit_exp(ctx, s + 2)
                        emit_pv(ctx, s)

                def emit_tail(ctx, fast_chain=False):
                    chunk, pair = ctx["chunk"], ctx["pair"]
                    pv_a, pv_b = ctx["pv_a"], ctx["pv_b"]
                    # evacuate pv (frees the PSUM banks for the next pair:
                    # ACT takes head A, DVE takes head B, in parallel)
                    pvs_a = pb.tile([HD + 1, 512], F32, tag="pvsa", bufs=1)
                    pvs_b = pb.tile([HD + 1, 512], F32, tag="pvsb", bufs=1)
                    nc.scalar.activation(pvs_a[:], pv_a[0:HD + 1, :], COPY)
                    nc.vector.tensor_copy(pvs_b[:], pv_b[0:HD + 1, :])
                    # denominator -> broadcast -> normalize -> cc_in
                    # (den rows reach partition 0 via DMA for the broadcast)
                    denr = pb.tile([1, 1024], F32, tag="denr", bufs=2)
                    nc.scalar.activation(denr[0:1, 0:512], pv_a[64:65, :], COPY)
                    nc.scalar.activation(denr[0:1, 512:1024], pv_b[64:65, :], COPY)
                    denb = pb.tile([64, 1024], F32, tag="denb", bufs=1)
                    nc.gpsimd.partition_broadcast(denb[:], denr[0:1, :], channels=64)
                    tma = pb.tile([64, 512], BF16, tag="tma", bufs=2)
                    tmb = pb.tile([64, 512], BF16, tag="tmb", bufs=2)
                    rep = pb.tile([64, 1024], F32, tag="rep", bufs=1)
                    nc.vector.reciprocal_approx_fast(out=rep[:], in_=denb[:])
                    nc.vector.tensor_mul(tma[:], pvs_a[0:64, :], rep[:, 0:512])
                    nc.vector.tensor_mul(tmb[:], pvs_b[0:64, :], rep[:, 512:1024])
                    r0 = pair * 128
                    nc.sync.dma_start(out=cc_in[chunk][r0:r0 + 64, :], in_=tma[:])
                    nc.sync.dma_start(out=cc_in[chunk][r0 + 64:r0 + 128, :], in_=tmb[:])

                def do_ag_chunk(chunk):
                    nc.gpsimd.collective_compute(
                        "AllGather",
                        mybir.AluOpType.bypass,
                        replica_groups=[[0, 1, 2, 3], [4, 5, 6, 7]],
                        ins=[cc_in[chunk][:].opt()],
                        outs=[cc_out[chunk][:].opt()],
                    )

                def emit_ag_load(chunk):
                    # one wide dispatch: [2048, 512] DRAM -> [128, 16, 512] SBUF
                    # (bass sprays large single descriptors across DMA rings).
                    # Dispatched from the GPSIMD queue: the sync queue runs
                    # ~25us ahead of the PE and would head-of-line block on the
                    # AllGather-done semaphore, stalling the swp/cc dispatches
                    # behind it; the gpsimd queue is paced by the per-pair
                    # partition broadcasts, so the AG is always done by then.
                    nc.gpsimd.dma_start(
                        out=agw[:].rearrange("p (m t) -> p m t", m=16),
                        in_=cc_out[chunk][:].rearrange("(m p) t -> p m t", p=128),
                    )

                def emit_ag_load_tt(chunk, tt):
                    # per-tt column gather: [128, 16, 128] -- lets the final wo
                    # groups chase the AllGather instead of one bulk reload
                    nc.sync.dma_start(
                        out=agw[:].rearrange("p (m tt f) -> p m tt f", m=16, tt=4)[
                            :, :, tt:tt + 1, :],
                        in_=cc_out[chunk][:].rearrange("(m p) (tt f) -> p m tt f",
                                                       p=128, tt=4)[:, :, tt:tt + 1, :],
                    )

                def emit_wo_group(chunk, tt, osb_w, evac_dve=False):
                    """one [128, 512] psum tile of out[:, chunk cols]"""
                    pso = bps.tile([128, 1024], F32, tag="qk2", bufs=3, name="pso")
                    for m in range(16):
                        nc.tensor.matmul(
                            pso[:, 0:512],
                            agw[:, m * 512 + tt * 128:m * 512 + (tt + 1) * 128],
                            wo_w[:, m * 512:(m + 1) * 512],
                            start=(m == 0), stop=(m == 15),
                        )
                    if evac_dve:
                        nc.vector.tensor_copy(osb_w[:, tt * 512:(tt + 1) * 512], pso[:, 0:512])
                    else:
                        nc.scalar.activation(osb_w[:, tt * 512:(tt + 1) * 512], pso[:, 0:512], COPY)

                def emit_out_store(chunk, osb_w):
                    tb = chunk * 512
                    nc.sync.dma_start(
                        out=out_d[tb:tb + 512, :].rearrange("(tt p) c -> p tt c", p=128),
                        in_=osb_w[:].rearrange("p (tt c) -> p tt c", tt=4),
                    )

                def emit_out_store_tt(chunk, tt, osb_w):
                    tb = chunk * 512
                    nc.sync.dma_start(
                        out=out_d[tb + tt * 128:tb + (tt + 1) * 128, :],
                        in_=osb_w[:, tt * 512:(tt + 1) * 512],
                    )

                # ---------- schedule ----------
                # fillers[(chunk, pair)] -> emitted between the previous
                # pair's tail and this pair's body.
                # Q-proj fillers: the matmul group runs in pair p's slot;
                # its RoPE (DVE) runs in pair p+1's slot so it never delays
                # the exp stream right after the projection.
                qr_stash = {}

                def q_mm_filler(q, dq):
                    def f():
                        qr_stash[(q, dq)] = emit_q_group(q, dq)
                    return f

                def q_rope_filler(q, dq):
                    def f():
                        emit_qrope_quarter(dq, q, qr_stash.pop((q, dq)))
                    return f

                def wo_dma_filler():
                    def f():
                        for hh in range(2):
                            nc.sync.dma_start(
                                out=wo_w[:, hh * 4096:(hh + 1) * 4096].rearrange(
                                    "p (m c) -> p m c", m=8
                                ),
                                in_=wo_d[hh * 1024:(hh + 1) * 1024, :].rearrange(
                                    "(m p) c -> p m c", p=128
                                ),
                            )
                    return f

                fillers = {}
                fillers[(0, 1)] = [wo_dma_filler()]
                for c in (0, 1, 2):
                    q = c + 1
                    fillers.setdefault((c, 0), []).append(q_mm_filler(q, 0))
                    fillers.setdefault((c, 1), []).append(q_rope_filler(q, 0))
                    fillers[(c, 1)].append(q_mm_filler(q, 1))
                    fillers[(c, 2)] = [q_rope_filler(q, 1), q_mm_filler(q, 2)]
                    fillers[(c, 3)] = [q_rope_filler(q, 2), q_mm_filler(q, 3)]

                pending = {}
                prev = None
                deferred_store = None
                for chunk in range(4):
                    for pair in range(4):
                        ctx = pending.pop((chunk, pair), None)
                        if ctx is None:
                            ctx = new_ctx(chunk, pair)
                            emit_prologue(ctx)
                        if prev is not None:
                            emit_tail(prev)
                            prev = None
                        # the previous boundary's output store dispatches here,
                        # one pair later, so the sync queue never blocks on the
                        # wo psum evacuations still in flight
                        if pair == 1 and deferred_store is not None:
                            emit_out_store(*deferred_store)
                            deferred_store = None
                        # gathered-chunk unload: late enough that the AllGather
                        # is done (no DMA-queue head blocking), early enough
                        # that the boundary wo block never waits
                        if chunk >= 1 and pair == 3:
                            emit_ag_load(chunk - 1)
                        for f in fillers.get((chunk, pair), []):
                            f()
                        emit_body(ctx)
                        prev = ctx
                    if chunk < 3:
                        nctx = new_ctx(chunk + 1, 0)
                        emit_prologue(nctx)
                        pending[(chunk + 1, 0)] = nctx
                    emit_tail(prev, fast_chain=(chunk == 3))
                    prev = None
                    do_ag_chunk(chunk)
                    if chunk < 3:
                        emit_qrope_quarter(3, chunk + 1, qr_stash.pop((chunk + 1, 3)))
                    # chunk-boundary wo block: all four groups of chunk-1's
                    # AllGather (a full chunk of slack) run here as PE filler
                    # while this chunk's AllGather transfers
                    if chunk >= 1:
                        osb_w = pb.tile([128, 4 * 512], F32, tag="osb", bufs=1,
                                        name=f"osb{chunk - 1}")
                        for tt in range(4):
                            emit_wo_group(chunk - 1, tt, osb_w, evac_dve=(tt % 2 == 1))
                        if chunk < 3:
                            deferred_store = (chunk - 1, osb_w)
                        else:
                            emit_out_store(chunk - 1, osb_w)
                # final chunk: per-tt column unloads let each wo group start as
                # soon as its own slice of the gather has been pulled back in;
                # the per-tt WAR on the chunk-2 wo reads also releases early
                for tt in range(4):
                    emit_ag_load_tt(3, tt)
                osb_w = pb.tile([128, 4 * 512], F32, tag="osb", bufs=1, name="osb3")
                for tt in range(4):
                    emit_wo_group(3, tt, osb_w, evac_dve=(tt % 2 == 1))
                    emit_out_store_tt(3, tt, osb_w)

    return nc


# ---------------------------------------------------------------------------
# Host side
# ---------------------------------------------------------------------------

_CACHE = {}


def _rope_tables():
    i = np.arange(32)
    freqs = 1.0 / (THETA ** (2.0 * i / HD))          # [32]
    ang = np.arange(T, dtype=np.float64)[:, None] * freqs[None, :]  # [T, 32]
    cos = np.cos(ang)
    sin = np.sin(ang)
    p = np.arange(128)
    fi = p % 32
    sign = np.where(p % 64 < 32, -1.0, 1.0)
    cosr = cos[:, fi].T                               # [128, T]
    sinr = (sin[:, fi] * sign[None, :]).T             # [128, T]
    return cosr.astype(np.float32), sinr.astype(np.float32)


def _colperm(n_heads):
    """rotate-half permutation: per 64-col head block, evens then odds"""
    blk = np.concatenate([np.arange(0, HD, 2), np.arange(1, HD, 2)])
    return np.concatenate([h * HD + blk for h in range(n_heads)])


def _prep_in_maps(x, wq, wk, wv, wo):
    cosr, sinr = _rope_tables()
    qperm = _colperm(32)
    kperm = _colperm(8)
    wq_p = (wq.astype(np.float64) / 8.0)[:, qperm]    # fold 1/sqrt(hd)
    wk_p = wk[:, kperm]
    in_maps = []
    for c in range(N_CORES):
        b, g = divmod(c, 4)
        in_maps.append({
            "xT": np.ascontiguousarray(x[b].T).astype(NPBF16),
            "wkvq": np.concatenate(
                [wk_p[:, g * DKV:(g + 1) * DKV], wv[:, g * DKV:(g + 1) * DKV],
                 wq_p[:, g * DQ:(g + 1) * DQ]],
                axis=1,
            ).astype(NPBF16),
            "wo": wo[:, g * DQ:(g + 1) * DQ].astype(NPBF16),
            "cosr": cosr.astype(NPBF16),
            "sinr": sinr.astype(NPBF16),
        })
    return in_maps


def get_nc():
    if "nc" not in _CACHE:
        nc = build_nc()
        if not nc.is_finalized():
            nc.finalize()
        _CACHE["nc"] = nc
    return _CACHE["nc"]


def run_on_hw(in_maps, trace=False):
    nc = get_nc()
    return run_bass_kernel_spmd(nc, in_maps, core_ids=list(range(N_CORES)), trace=trace)


def _assemble(results):
    out = np.zeros((2, T, C), dtype=np.float32)
    for c in range(N_CORES):
        b, g = divmod(c, 4)
        out[b][:, g * DQ:(g + 1) * DQ] = np.asarray(results[c]["out"], dtype=np.float32)
    return out


def kernel(x, wq, wk, wv, wo):
    in_maps = _prep_in_maps(
        np.asarray(x, np.float32), np.asarray(wq, np.float32),
        np.asarray(wk, np.float32), np.asarray(wv, np.float32),
        np.asarray(wo, np.float32),
    )
    res = run_on_hw(in_maps, trace=False)
    return _assemble(res.results)


# revision 29
# speedup vs baseline: 1.0245x; 1.0051x over previous
"""Trainium2 Bass kernel for GQA attention with RoPE (dense_transformer).

Reference computation (per batch b):
    q = x @ wq  -> [T, 32, 64],  k = x @ wk -> [T, 8, 64], v = x @ wv
    rope(q), rope(k); scores = q k^T / 8; w = softmax(scores); out = (w v) @ wo

Sharding over 8 NeuronCores: 2 batch groups x 4-way head tensor parallel.
Core c: batch b=c//4, head group g=c%4 (q-heads 8g..8g+8, kv-heads 2g,2g+1).
Within a group of 4 cores the attention outputs (transposed, [512,T]) are
AllGather'd per 512-column t-chunk; each core then computes a 512-column
slice of out = attn @ wo.

v4 schedule — v3 merged pipeline plus:
  - Batched DMA dispatch: the sync engine serializes dma_start dispatch
    at ~600ns each, so the 16 per-chunk AllGather unloads become 1 wide
    dispatch, wo weights 2, x^T quarters 1-3 use 4 each, and the 4
    per-chunk output stores become 1 (the AG latency itself is a ~16-25us
    fixed cost regardless of payload, so splitting collectives does not
    help; batching the dispatches removes the sync-queue head-of-line
    blocking that stalled the PE mid-kernel).
  - Chunk 3 endgame is a per-tt chase: 4 column-sliced unloads of the
    last gather, each wo group starting as soon as its slice lands, with
    per-tt output stores and ACT/DVE alternating psum evacuation.
"""

import numpy as np
import ml_dtypes

import concourse.bass as bass
import concourse.mybir as mybir
import concourse.tile as tile
from concourse import bacc
from concourse.bass_utils import run_bass_kernel_spmd

BF16 = mybir.dt.bfloat16
F32 = mybir.dt.float32
I16 = mybir.dt.int16

T = 2048          # sequence length (also s dim)
C = 2048          # model dim
HD = 64           # head dim
DQ = 512          # q dims per core (8 heads)
DKV = 128         # kv dims per core (2 kv heads)
N_CORES = 8
THETA = 10000.0

EXP = mybir.ActivationFunctionType.Exp
COPY = mybir.ActivationFunctionType.Copy
MULT = mybir.AluOpType.mult
ADD = mybir.AluOpType.add

# Schraudolph exp producing bf16 BITS via one DVE tensor_scalar:
# bf16_bits(e^x) ~= int16(x * 128/ln2 + (127<<7) - 0.0579*128)
EXP_A = 128.0 / float(np.log(2.0))
EXP_B = 16256.0 - 0.0579 * 128.0
NPBF16 = ml_dtypes.bfloat16


def build_nc():
    nc = bacc.Bacc()

    xT_d = nc.declare_dram_parameter("xT", [C, T], BF16, isOutput=False)
    wkvq_d = nc.declare_dram_parameter("wkvq", [C, 2 * DKV + DQ], BF16, isOutput=False)
    wo_d = nc.declare_dram_parameter("wo", [C, DQ], BF16, isOutput=False)
    cosr_d = nc.declare_dram_parameter("cosr", [128, T], BF16, isOutput=False)
    sinr_d = nc.declare_dram_parameter("sinr", [128, T], BF16, isOutput=False)
    out_d = nc.declare_dram_parameter("out", [T, DQ], F32, isOutput=True)

    with tile.TileContext(nc) as tc:
        with (
            tc.tile_pool(name="persist", bufs=1) as pp,
            tc.tile_pool(name="dram", bufs=1, space="DRAM") as dp,
        ):
            # ---------- persistent SBUF ----------
            # roped Q^T tiles: qt[p] holds local heads (2p, 2p+1) on partitions
            # [0:64] / [64:128]; free dim = t
            qt = [pp.tile([128, T], BF16, tag=f"qt{i}", name=f"qt{i}") for i in range(4)]
            # duplicated roped K^T tiles: ktd[j] = [kv_j ; kv_j]
            ktd = [pp.tile([128, T], BF16, tag=f"ktd{i}", name=f"ktd{i}") for i in range(2)]
            # V augmented with a ones column, padded to 128 stationary cols
            # (full-width weights enable fast-weight-load):
            # per kv head, per s-tile [128, 128] = [v(64) | ones | zeros]
            vaug = [
                [pp.tile([128, 128], BF16, tag=f"va{j}_{s}", name=f"va{j}_{s}") for s in range(16)]
                for j in range(2)
            ]
            cosr = pp.tile([128, T], BF16, tag="cosr")
            sinr = pp.tile([128, T], BF16, tag="sinr")
            # wo weights, one wide tile; 512-col group j holds the wo rows for
            # gathered d-tile j (host pre-permutes rows into half-AG order)
            wo_w = pp.tile([128, 16 * 512], BF16, tag="wo_w", name="wo_w")
            # gathered attention: [128, 16*512]; col group m = d-tile m
            agw = pp.tile([128, 16 * 512], BF16, tag="agw", name="agw")

            for j in range(2):
                for s in range(16):
                    nc.gpsimd.memset(vaug[j][s][:, HD + 1:], 0.0)
                    nc.gpsimd.memset(vaug[j][s][:, HD:HD + 1], 1.0)
            # warm the ACT exp table set early so the ~2.7us ACT_TABLE_LOAD is
            # off the attention critical path
            warm = pp.tile([1, 8], F32, tag="warm")
            nc.gpsimd.memset(warm[:], 0.0)
            nc.scalar.activation(warm[:], warm[:], EXP)
            # ones stationary for the PE denominator broadcast (K=1 matmul)
            ones1 = pp.tile([1, 64], BF16, tag="ones1")
            nc.gpsimd.memset(ones1[:], 1.0)

            # ---------- DRAM bounce for AllGather (4 chunks of 512 t) ----------
            cc_in = [dp.tile([DQ, 512], BF16, tag=f"cci{i}", name=f"cci{i}") for i in range(4)]
            cc_out = [dp.tile([4 * DQ, 512], BF16, tag=f"cco{i}", name=f"cco{i}") for i in range(4)]
            # warmup collective: absorbs the DGE start delay (~11us) and the
            # initial cross-core sync skew so the first real AllGather is fast
            cw_in = dp.tile([128, 16], BF16, tag="cwi", name="cwi")
            cw_out = dp.tile([512, 16], BF16, tag="cwo", name="cwo")

            with (
                tc.tile_pool(name="pb", bufs=1) as pb,
                tc.tile_pool(name="pb_ps", bufs=1, space=bass.MemorySpace.PSUM) as bps,
            ):
                wkvq_sb = [
                    pb.tile([128, 2 * DKV + DQ], BF16, tag=f"wkvq{i}", name=f"wkvq{i}")
                    for i in range(16)
                ]
                wk_sb = [t[:, 0:DKV] for t in wkvq_sb]
                wv_sb = [t[:, DKV:2 * DKV] for t in wkvq_sb]
                wq_sb = [t[:, 2 * DKV:2 * DKV + DQ] for t in wkvq_sb]
                ktraw = pb.tile([128, T], BF16, tag="ktraw")

                # ---------- warmups ----------
                junk = pb.tile([128, 512], BF16, tag="junk")
                nc.gpsimd.memset(junk[:], 0.0)
                nc.sync.dma_start(out=cw_in[:], in_=junk[:, 0:16])
                nc.gpsimd.collective_compute(
                    "AllGather",
                    mybir.AluOpType.bypass,
                    replica_groups=[[0, 1, 2, 3], [4, 5, 6, 7]],
                    ins=[cw_in[:].opt()],
                    outs=[cw_out[:].opt()],
                )
                for _ in range(5):
                    jps = bps.tile([128, 1024], F32, tag="qk2", bufs=3, name="jps")
                    nc.tensor.matmul(jps[:, 0:512], junk[:, 0:128], junk[:], start=True, stop=True)

                # ---------- x^T tiles ----------
                # wide per-quarter tiles [128, 16*512]; col group kc holds
                # xT[kc*128:(kc+1)*128, q*512:(q+1)*512].  bufs=3: quarter q+3
                # reuses quarter q's slot once its last consumer retires.
                xtw = {}

                def xt(kc, q):
                    return xtw[q][:, kc * 512:(kc + 1) * 512]

                def emit_xtq_dma(q):
                    t = pb.tile([128, 16 * 512], BF16, tag="xtw", bufs=3, name=f"xtw_{q}")
                    xtw[q] = t
                    # 4 dispatches of 4 kc-tiles each
                    for g in range(4):
                        nc.sync.dma_start(
                            out=t[:, g * 2048:(g + 1) * 2048].rearrange(
                                "p (kc t) -> p kc t", kc=4
                            ),
                            in_=xT_d[g * 512:(g + 1) * 512,
                                     q * 512:(q + 1) * 512].rearrange(
                                "(kc p) t -> p kc t", p=128
                            ),
                        )

                # quarter 0 as 16 separate dispatches (all DMA rings in
                # parallel -- latency-critical), interleaved kc-major with the
                # weights so the K-proj chain's operands arrive consumer-order
                xtw[0] = pb.tile([128, 16 * 512], BF16, tag="xtw", bufs=3, name="xtw_0")
                for kc in range(16):
                    nc.sync.dma_start(
                        out=wkvq_sb[kc][:], in_=wkvq_d[kc * 128:(kc + 1) * 128, :]
                    )
                    nc.sync.dma_start(
                        out=xtw[0][:, kc * 512:(kc + 1) * 512],
                        in_=xT_d[kc * 128:(kc + 1) * 128, 0:512],
                    )
                emit_xtq_dma(1)
                nc.sync.dma_start(out=cosr[:], in_=cosr_d[:])
                nc.sync.dma_start(out=sinr[:], in_=sinr_d[:])
                # xtw q2/q3 and wo stream in during the prologue tail / chunk
                # 0 via deferred emission (the DMA dispatch queue is serial at
                # ~600ns per descriptor: consumer order = dispatch order)

                # ---------- projection building blocks ----------
                def emit_k_quarter(q):
                    ps = bps.tile([128, 1024], F32, tag="qk2", bufs=3, name="kps")
                    for kc in range(16):
                        nc.tensor.matmul(
                            ps[:, 0:512], wk_sb[kc], xt(kc, q),
                            start=(kc == 0), stop=(kc == 15),
                        )
                    nc.vector.tensor_copy(ktraw[:, q * 512:(q + 1) * 512], ps[:, 0:512])

                def emit_v_slice(s):
                    q, qo = s // 4, (s % 4) * 128
                    psv = bps.tile([128, 512], F32, tag="pv", bufs=2, name="vps")
                    for kc in range(16):
                        nc.tensor.matmul(
                            psv[:, 0:128], xt(kc, q)[:, qo:qo + 128], wv_sb[kc],
                            start=(kc == 0), stop=(kc == 15),
                        )
                    nc.vector.tensor_copy(vaug[0][s][:, 0:HD], psv[:, 0:HD])
                    nc.vector.tensor_copy(vaug[1][s][:, 0:HD], psv[:, HD:2 * HD])

                def emit_krope_half(half):
                    # K rope on a [128,1024] half -> duplicated halves of ktd
                    t0 = half * 1024
                    ktr = pb.tile([128, 1024], BF16, tag="ktr", bufs=1)
                    swp = pb.tile([128, 1024], BF16, tag="swpk", bufs=1)
                    for a, b in ((0, 32), (32, 0), (64, 96), (96, 64)):
                        nc.sync.dma_start(out=swp[a:a + 32, :], in_=ktraw[b:b + 32, t0:t0 + 1024])
                    t1k = pb.tile([128, 1024], BF16, tag="t1k", bufs=1)
                    t2k = pb.tile([128, 1024], BF16, tag="t2k", bufs=1)
                    nc.vector.tensor_mul(t1k[:], ktraw[:, t0:t0 + 1024], cosr[:, t0:t0 + 1024])
                    nc.vector.tensor_mul(t2k[:], swp[:], sinr[:, t0:t0 + 1024])
                    nc.vector.tensor_add(ktr[:], t1k[:], t2k[:])
                    nc.sync.dma_start(out=ktd[0][0:64, t0:t0 + 1024], in_=ktr[0:64, :])
                    nc.sync.dma_start(out=ktd[0][64:128, t0:t0 + 1024], in_=ktr[0:64, :])
                    nc.sync.dma_start(out=ktd[1][0:64, t0:t0 + 1024], in_=ktr[64:128, :])
                    nc.sync.dma_start(out=ktd[1][64:128, t0:t0 + 1024], in_=ktr[64:128, :])

                def emit_q_group(q, dq):
                    """Q proj psum tile (dq, quarter q); evac on ACT."""
                    ps = bps.tile([128, 1024], F32, tag="qk2", bufs=3, name="qps")
                    for kc in range(16):
                        nc.tensor.matmul(
                            ps[:, 0:512],
                            wkvq_sb[kc][:, 2 * DKV + dq * 128:2 * DKV + (dq + 1) * 128],
                            xt(kc, q),
                            start=(kc == 0), stop=(kc == 15),
                        )
                    qr = pb.tile([128, 512], BF16, tag="qraw", bufs=3, name="qr")
                    nc.scalar.activation(qr[:], ps[:, 0:512], COPY)
                    return qr

                def emit_qrope_quarter(dq, q, qr):
                    # qt[dq][:, q-quarter] = qr*cosr + swap32(qr)*sinr
                    t0 = q * 512
                    swp = pb.tile([128, 512], BF16, tag="swpq", bufs=2)
                    for a, b in ((0, 32), (32, 0), (64, 96), (96, 64)):
                        nc.sync.dma_start(out=swp[a:a + 32, :], in_=qr[b:b + 32, :])
                    t1 = pb.tile([128, 512], BF16, tag="t1q", bufs=2)
                    t2 = pb.tile([128, 512], BF16, tag="t2q", bufs=2)
                    nc.vector.tensor_mul(t1[:], qr[:], cosr[:, t0:t0 + 512])
                    nc.vector.tensor_mul(t2[:], swp[:], sinr[:, t0:t0 + 512])
                    nc.vector.tensor_add(qt[dq][:, t0:t0 + 512], t1[:], t2[:])

                # ---------- prologue ----------
                # Q quarter 0 goes early so its xtw slot frees for quarter 3,
                # letting K finish (and ktd fully write) before the trailing
                # V block -- the first attention QK then starts stall-free.
                emit_k_quarter(0)
                emit_k_quarter(1)
                for s in range(4):
                    emit_v_slice(s)
                emit_krope_half(0)
                for dq in range(4):
                    qr = emit_q_group(0, dq)
                    emit_qrope_quarter(dq, 0, qr)
                emit_xtq_dma(2)
                emit_xtq_dma(3)
                for s in range(4, 8):
                    emit_v_slice(s)
                emit_k_quarter(2)
                emit_k_quarter(3)
                emit_krope_half(1)
                for s in range(8, 16):
                    emit_v_slice(s)

                # ---------- attention pair blocks ----------
                def new_ctx(chunk, pair):
                    return {
                        "chunk": chunk, "pair": pair,
                        "ta": chunk * 512, "kv": pair // 2,
                        "pv_a": bps.tile([128, 512], F32, tag="pv", bufs=2, name="pv_a"),
                        "pv_b": bps.tile([128, 512], F32, tag="pv", bufs=2, name="pv_b"),
                        "qks": {}, "ess": {},
                    }

                def emit_qk(ctx, s):
                    qk = bps.tile([128, 1024], F32, tag="qk2", bufs=3)
                    ctx["qks"][s] = qk
                    kv, pair, ta = ctx["kv"], ctx["pair"], ctx["ta"]
                    # row-packed pair: head A on rows 0-63 -> tile (0,0),
                    # head B on rows 64-127 -> tile (64,0): concurrent MMs
                    nc.tensor.matmul(
                        qk[:, 0:512],
                        ktd[kv][0:64, s * 128:(s + 1) * 128],
                        qt[pair][0:64, ta:ta + 512],
                        start=True, stop=True,
                    )
                    nc.tensor.matmul(
                        qk[:, 512:1024],
                        ktd[kv][64:128, s * 128:(s + 1) * 128],
                        qt[pair][64:128, ta:ta + 512],
                        start=True, stop=True,
                    )

                def emit_exp(ctx, s):
                    # full-tile exp alternating engines per s (one dispatch
                    # bubble per 1024 cols); bufs=4 (even) so each pool slot
                    # is always rewritten by the SAME engine
                    qk = ctx["qks"][s]
                    es = pb.tile([128, 1024], BF16, tag="es", bufs=4)
                    ctx["ess"][s] = es
                    if s % 2 == 0:
                        nc.scalar.activation(es[:], qk[:], EXP)
                    else:
                        nc.vector.tensor_scalar(
                            es[:].bitcast(I16), qk[:],
                            EXP_A, EXP_B, MULT, ADD,
                        )

                def emit_pv(ctx, s):
                    es = ctx["ess"].pop(s)
                    ctx["qks"].pop(s)
                    kv = ctx["kv"]
                    nc.tensor.matmul(
                        ctx["pv_a"][:], vaug[kv][s][:], es[:, 0:512],
                        start=(s == 0), stop=(s == 15),
                        skip_group_check=True,
                    )
                    nc.tensor.matmul(
                        ctx["pv_b"][:], vaug[kv][s][:], es[:, 512:1024],
                        start=(s == 0), stop=(s == 15),
                        skip_group_check=True,
                    )

                def emit_prologue(ctx):
                    # 2-deep lookahead: QK runs two iterations ahead of PV so
                    # the exp latency stays off the PE critical path; both
                    # first exps precede the previous pair's tail in engine
                    # queue order
                    emit_qk(ctx, 0)
                    emit_qk(ctx, 1)
                    emit_exp(ctx, 0)
                    emit_exp(ctx, 1)

                def emit_body(ctx):
                    for s in range(16):
                        if s + 2 < 16:
                            emit_qk(ctx, s + 2)
                            emit_exp(ctx, s + 2)
                        emit_pv(ctx, s)

                def emit_tail(ctx, fast_chain=False):
                    chunk, pair = ctx["chunk"], ctx["pair"]
                    pv_a, pv_b = ctx["pv_a"], ctx["pv_b"]
                    # evacuate pv (frees the PSUM banks for the next pair:
                    # ACT takes head A, DVE takes head B, in parallel)
                    pvs_a = pb.tile([HD + 1, 512], F32, tag="pvsa", bufs=1)
                    pvs_b = pb.tile([HD + 1, 512], F32, tag="pvsb", bufs=1)
                    nc.scalar.activation(pvs_a[:], pv_a[0:HD + 1, :], COPY)
                    nc.vector.tensor_copy(pvs_b[:], pv_b[0:HD + 1, :])
                    # denominator -> PE broadcast -> normalize -> cc_in.
                    # The broadcast is a K=1 ones-stationary matmul into the
                    # just-freed pv psum slots: partition_broadcast would ride
                    # the gpsimd queue, which collective_compute blocks for its
                    # whole ~25us duration, flakily stalling the tails.
                    denr = pb.tile([1, 1024], BF16, tag="denr", bufs=2)
                    nc.scalar.activation(denr[0:1, 0:512], pv_a[64:65, :], COPY)
                    nc.scalar.activation(denr[0:1, 512:1024], pv_b[64:65, :], COPY)
                    bc_a = bps.tile([128, 512], F32, tag="pv", bufs=2, name="bc_a")
                    bc_b = bps.tile([128, 512], F32, tag="pv", bufs=2, name="bc_b")
                    nc.tensor.matmul(bc_a[0:64, :], ones1[0:1, :], denr[0:1, 0:512],
                                     start=True, stop=True)
                    nc.tensor.matmul(bc_b[0:64, :], ones1[0:1, :], denr[0:1, 512:1024],
                                     start=True, stop=True)
                    tma = pb.tile([64, 512], BF16, tag="tma", bufs=2)
                    tmb = pb.tile([64, 512], BF16, tag="tmb", bufs=2)
                    rep = pb.tile([64, 1024], F32, tag="rep", bufs=1)
                    nc.vector.reciprocal_approx_fast(out=rep[:, 0:512], in_=bc_a[0:64, :])
                    nc.vector.reciprocal_approx_fast(out=rep[:, 512:1024], in_=bc_b[0:64, :])
                    nc.vector.tensor_mul(tma[:], pvs_a[0:64, :], rep[:, 0:512])
                    nc.vector.tensor_mul(tmb[:], pvs_b[0:64, :], rep[:, 512:1024])
                    r0 = pair * 128
                    nc.sync.dma_start(out=cc_in[chunk][r0:r0 + 64, :], in_=tma[:])
                    nc.sync.dma_start(out=cc_in[chunk][r0 + 64:r0 + 128, :], in_=tmb[:])

                def do_ag_chunk(chunk):
                    nc.gpsimd.collective_compute(
                        "AllGather",
                        mybir.AluOpType.bypass,
                        replica_groups=[[0, 1, 2, 3], [4, 5, 6, 7]],
                        ins=[cc_in[chunk][:].opt()],
                        outs=[cc_out[chunk][:].opt()],
                    )

                def emit_ag_load(chunk):
                    # one wide dispatch: [2048, 512] DRAM -> [128, 16, 512] SBUF
                    # (bass sprays large single descriptors across DMA rings).
                    # Dispatched from the GPSIMD queue: the sync queue runs
                    # ~25us ahead of the PE and would head-of-line block on the
                    # AllGather-done semaphore, stalling the swp/cc dispatches
                    # behind it; the gpsimd queue is paced by the per-pair
                    # partition broadcasts, so the AG is always done by then.
                    nc.gpsimd.dma_start(
                        out=agw[:].rearrange("p (m t) -> p m t", m=16),
                        in_=cc_out[chunk][:].rearrange("(m p) t -> p m t", p=128),
                    )

                def emit_ag_load_tt(chunk, tt):
                    # per-tt column gather: [128, 16, 128] -- lets the final wo
                    # groups chase the AllGather instead of one bulk reload
                    nc.sync.dma_start(
                        out=agw[:].rearrange("p (m tt f) -> p m tt f", m=16, tt=4)[
                            :, :, tt:tt + 1, :],
                        in_=cc_out[chunk][:].rearrange("(m p) (tt f) -> p m tt f",
                                                       p=128, tt=4)[:, :, tt:tt + 1, :],
                    )

                def emit_wo_group(chunk, tt, osb_w, evac_dve=False):
                    """one [128, 512] psum tile of out[:, chunk cols]"""
                    pso = bps.tile([128, 1024], F32, tag="qk2", bufs=3, name="pso")
                    for m in range(16):
                        nc.tensor.matmul(
                            pso[:, 0:512],
                            agw[:, m * 512 + tt * 128:m * 512 + (tt + 1) * 128],
                            wo_w[:, m * 512:(m + 1) * 512],
                            start=(m == 0), stop=(m == 15),
                        )
                    if evac_dve:
                        nc.vector.tensor_copy(osb_w[:, tt * 512:(tt + 1) * 512], pso[:, 0:512])
                    else:
                        nc.scalar.activation(osb_w[:, tt * 512:(tt + 1) * 512], pso[:, 0:512], COPY)

                def emit_out_store(chunk, osb_w):
                    tb = chunk * 512
                    nc.sync.dma_start(
                        out=out_d[tb:tb + 512, :].rearrange("(tt p) c -> p tt c", p=128),
                        in_=osb_w[:].rearrange("p (tt c) -> p tt c", tt=4),
                    )

                def emit_out_store_tt(chunk, tt, osb_w):
                    tb = chunk * 512
                    nc.sync.dma_start(
                        out=out_d[tb + tt * 128:tb + (tt + 1) * 128, :],
                        in_=osb_w[:, tt * 512:(tt + 1) * 512],
                    )

                # ---------- schedule ----------
                # fillers[(chunk, pair)] -> emitted between the previous
                # pair's tail and this pair's body.
                # Q-proj fillers: the matmul group runs in pair p's slot;
                # its RoPE (DVE) runs in pair p+1's slot so it never delays
                # the exp stream right after the projection.
                qr_stash = {}

                def q_mm_filler(q, dq):
                    def f():
                        qr_stash[(q, dq)] = emit_q_group(q, dq)
                    return f

                def q_rope_filler(q, dq):
                    def f():
                        emit_qrope_quarter(dq, q, qr_stash.pop((q, dq)))
                    return f

                def wo_dma_filler():
                    def f():
                        for hh in range(2):
                            nc.sync.dma_start(
                                out=wo_w[:, hh * 4096:(hh + 1) * 4096].rearrange(
                                    "p (m c) -> p m c", m=8
                                ),
                                in_=wo_d[hh * 1024:(hh + 1) * 1024, :].rearrange(
                                    "(m p) c -> p m c", p=128
                                ),
                            )
                    return f

                fillers = {}
                fillers[(0, 1)] = [wo_dma_filler()]
                for c in (0, 1, 2):
                    q = c + 1
                    fillers.setdefault((c, 0), []).append(q_mm_filler(q, 0))
                    fillers.setdefault((c, 1), []).append(q_rope_filler(q, 0))
                    fillers[(c, 1)].append(q_mm_filler(q, 1))
                    fillers[(c, 2)] = [q_rope_filler(q, 1), q_mm_filler(q, 2)]
                    fillers[(c, 3)] = [q_rope_filler(q, 2), q_mm_filler(q, 3)]

                pending = {}
                prev = None
                deferred_store = None
                for chunk in range(4):
                    for pair in range(4):
                        ctx = pending.pop((chunk, pair), None)
                        if ctx is None:
                            ctx = new_ctx(chunk, pair)
                            emit_prologue(ctx)
                        if prev is not None:
                            emit_tail(prev)
                            prev = None
                        # the previous boundary's output store dispatches here,
                        # one pair later, so the sync queue never blocks on the
                        # wo psum evacuations still in flight
                        if pair == 1 and deferred_store is not None:
                            emit_out_store(*deferred_store)
                            deferred_store = None
                        # gathered-chunk unload: late enough that the AllGather
                        # is done (no DMA-queue head blocking), early enough
                        # that the boundary wo block never waits
                        if chunk >= 1 and pair == 3:
                            emit_ag_load(chunk - 1)
                        for f in fillers.get((chunk, pair), []):
                            f()
                        emit_body(ctx)
                        prev = ctx
                    if chunk < 3:
                        nctx = new_ctx(chunk + 1, 0)
                        emit_prologue(nctx)
                        pending[(chunk + 1, 0)] = nctx
                    emit_tail(prev, fast_chain=(chunk == 3))
                    prev = None
                    do_ag_chunk(chunk)
                    if chunk < 3:
                        emit_qrope_quarter(3, chunk + 1, qr_stash.pop((chunk + 1, 3)))
                    # chunk-boundary wo block: all four groups of chunk-1's
                    # AllGather (a full chunk of slack) run here as PE filler
                    # while this chunk's AllGather transfers
                    if chunk >= 1:
                        osb_w = pb.tile([128, 4 * 512], F32, tag="osb", bufs=1,
                                        name=f"osb{chunk - 1}")
                        for tt in range(4):
                            emit_wo_group(chunk - 1, tt, osb_w, evac_dve=(tt % 2 == 1))
                        if chunk < 3:
                            deferred_store = (chunk - 1, osb_w)
                        else:
                            emit_out_store(chunk - 1, osb_w)
                # final chunk: per-tt column unloads let each wo group start as
                # soon as its own slice of the gather has been pulled back in;
                # the per-tt WAR on the chunk-2 wo reads also releases early
                for tt in range(4):
                    emit_ag_load_tt(3, tt)
                osb_w = pb.tile([128, 4 * 512], F32, tag="osb", bufs=1, name="osb3")
                for tt in range(4):
                    emit_wo_group(3, tt, osb_w, evac_dve=(tt % 2 == 1))
                    emit_out_store_tt(3, tt, osb_w)

    return nc


# ---------------------------------------------------------------------------
# Host side
# ---------------------------------------------------------------------------

_CACHE = {}


def _rope_tables():
    i = np.arange(32)
    freqs = 1.0 / (THETA ** (2.0 * i / HD))          # [32]
    ang = np.arange(T, dtype=np.float64)[:, None] * freqs[None, :]  # [T, 32]
    cos = np.cos(ang)
    sin = np.sin(ang)
    p = np.arange(128)
    fi = p % 32
    sign = np.where(p % 64 < 32, -1.0, 1.0)
    cosr = cos[:, fi].T                               # [128, T]
    sinr = (sin[:, fi] * sign[None, :]).T             # [128, T]
    return cosr.astype(np.float32), sinr.astype(np.float32)


def _colperm(n_heads):
    """rotate-half permutation: per 64-col head block, evens then odds"""
    blk = np.concatenate([np.arange(0, HD, 2), np.arange(1, HD, 2)])
    return np.concatenate([h * HD + blk for h in range(n_heads)])


def _prep_in_maps(x, wq, wk, wv, wo):
    cosr, sinr = _rope_tables()
    qperm = _colperm(32)
    kperm = _colperm(8)
    wq_p = (wq.astype(np.float64) / 8.0)[:, qperm]    # fold 1/sqrt(hd)
    wk_p = wk[:, kperm]
    in_maps = []
    for c in range(N_CORES):
        b, g = divmod(c, 4)
        in_maps.append({
            "xT": np.ascontiguousarray(x[b].T).astype(NPBF16),
            "wkvq": np.concatenate(
                [wk_p[:, g * DKV:(g + 1) * DKV], wv[:, g * DKV:(g + 1) * DKV],
                 wq_p[:, g * DQ:(g + 1) * DQ]],
                axis=1,
            ).astype(NPBF16),
            "wo": wo[:, g * DQ:(g + 1) * DQ].astype(NPBF16),
            "cosr": cosr.astype(NPBF16),
            "sinr": sinr.astype(NPBF16),
        })
    return in_maps


def get_nc():
    if "nc" not in _CACHE:
        nc = build_nc()
        if not nc.is_finalized():
            nc.finalize()
        _CACHE["nc"] = nc
    return _CACHE["nc"]


def run_on_hw(in_maps, trace=False):
    nc = get_nc()
    return run_bass_kernel_spmd(nc, in_maps, core_ids=list(range(N_CORES)), trace=trace)


def _assemble(results):
    out = np.zeros((2, T, C), dtype=np.float32)
    for c in range(N_CORES):
        b, g = divmod(c, 4)
        out[b][:, g * DQ:(g + 1) * DQ] = np.asarray(results[c]["out"], dtype=np.float32)
    return out


def kernel(x, wq, wk, wv, wo):
    in_maps = _prep_in_maps(
        np.asarray(x, np.float32), np.asarray(wq, np.float32),
        np.asarray(wk, np.float32), np.asarray(wv, np.float32),
        np.asarray(wo, np.float32),
    )
    res = run_on_hw(in_maps, trace=False)
    return _assemble(res.results)


# revision 30
# speedup vs baseline: 1.0253x; 1.0007x over previous
"""Trainium2 Bass kernel for GQA attention with RoPE (dense_transformer).

Reference computation (per batch b):
    q = x @ wq  -> [T, 32, 64],  k = x @ wk -> [T, 8, 64], v = x @ wv
    rope(q), rope(k); scores = q k^T / 8; w = softmax(scores); out = (w v) @ wo

Sharding over 8 NeuronCores: 2 batch groups x 4-way head tensor parallel.
Core c: batch b=c//4, head group g=c%4 (q-heads 8g..8g+8, kv-heads 2g,2g+1).
Within a group of 4 cores the attention outputs (transposed, [512,T]) are
AllGather'd per 512-column t-chunk; each core then computes a 512-column
slice of out = attn @ wo.

v4 schedule — v3 merged pipeline plus:
  - Batched DMA dispatch: the sync engine serializes dma_start dispatch
    at ~600ns each, so the 16 per-chunk AllGather unloads become 1 wide
    dispatch, wo weights 2, x^T quarters 1-3 use 4 each, and the 4
    per-chunk output stores become 1 (the AG latency itself is a ~16-25us
    fixed cost regardless of payload, so splitting collectives does not
    help; batching the dispatches removes the sync-queue head-of-line
    blocking that stalled the PE mid-kernel).
  - Chunk 3 endgame is a per-tt chase: 4 column-sliced unloads of the
    last gather, each wo group starting as soon as its slice lands, with
    per-tt output stores and ACT/DVE alternating psum evacuation.
  - collective_compute blocks the gpsimd queue for its whole ~25us
    duration, so nothing tail-critical may ride that queue: the softmax
    denominator broadcast is a K=1 ones-stationary PE matmul into the
    just-freed pv psum slots (not partition_broadcast), and the gather
    unloads dispatch from gpsimd (paced behind the collectives they wait
    on) rather than the sync queue (which runs ~25us ahead of the PE and
    would head-of-line block the swp/cc dispatches).
"""

import numpy as np
import ml_dtypes

import concourse.bass as bass
import concourse.mybir as mybir
import concourse.tile as tile
from concourse import bacc
from concourse.bass_utils import run_bass_kernel_spmd

BF16 = mybir.dt.bfloat16
F32 = mybir.dt.float32
I16 = mybir.dt.int16

T = 2048          # sequence length (also s dim)
C = 2048          # model dim
HD = 64           # head dim
DQ = 512          # q dims per core (8 heads)
DKV = 128         # kv dims per core (2 kv heads)
N_CORES = 8
THETA = 10000.0

EXP = mybir.ActivationFunctionType.Exp
COPY = mybir.ActivationFunctionType.Copy
MULT = mybir.AluOpType.mult
ADD = mybir.AluOpType.add

# Schraudolph exp producing bf16 BITS via one DVE tensor_scalar:
# bf16_bits(e^x) ~= int16(x * 128/ln2 + (127<<7) - 0.0579*128)
EXP_A = 128.0 / float(np.log(2.0))
EXP_B = 16256.0 - 0.0579 * 128.0
NPBF16 = ml_dtypes.bfloat16


def build_nc():
    nc = bacc.Bacc()

    xT_d = nc.declare_dram_parameter("xT", [C, T], BF16, isOutput=False)
    wkvq_d = nc.declare_dram_parameter("wkvq", [C, 2 * DKV + DQ], BF16, isOutput=False)
    wo_d = nc.declare_dram_parameter("wo", [C, DQ], BF16, isOutput=False)
    cosr_d = nc.declare_dram_parameter("cosr", [128, T], BF16, isOutput=False)
    sinr_d = nc.declare_dram_parameter("sinr", [128, T], BF16, isOutput=False)
    out_d = nc.declare_dram_parameter("out", [T, DQ], F32, isOutput=True)

    with tile.TileContext(nc) as tc:
        with (
            tc.tile_pool(name="persist", bufs=1) as pp,
            tc.tile_pool(name="dram", bufs=1, space="DRAM") as dp,
        ):
            # ---------- persistent SBUF ----------
            # roped Q^T tiles: qt[p] holds local heads (2p, 2p+1) on partitions
            # [0:64] / [64:128]; free dim = t
            qt = [pp.tile([128, T], BF16, tag=f"qt{i}", name=f"qt{i}") for i in range(4)]
            # duplicated roped K^T tiles: ktd[j] = [kv_j ; kv_j]
            ktd = [pp.tile([128, T], BF16, tag=f"ktd{i}", name=f"ktd{i}") for i in range(2)]
            # V augmented with a ones column, padded to 128 stationary cols
            # (full-width weights enable fast-weight-load):
            # per kv head, per s-tile [128, 128] = [v(64) | ones | zeros]
            vaug = [
                [pp.tile([128, 128], BF16, tag=f"va{j}_{s}", name=f"va{j}_{s}") for s in range(16)]
                for j in range(2)
            ]
            cosr = pp.tile([128, T], BF16, tag="cosr")
            sinr = pp.tile([128, T], BF16, tag="sinr")
            # wo weights, one wide tile; 512-col group j holds the wo rows for
            # gathered d-tile j (host pre-permutes rows into half-AG order)
            wo_w = pp.tile([128, 16 * 512], BF16, tag="wo_w", name="wo_w")
            # gathered attention: [128, 16*512]; col group m = d-tile m
            agw = pp.tile([128, 16 * 512], BF16, tag="agw", name="agw")

            for j in range(2):
                for s in range(16):
                    nc.gpsimd.memset(vaug[j][s][:, HD + 1:], 0.0)
                    nc.gpsimd.memset(vaug[j][s][:, HD:HD + 1], 1.0)
            # warm the ACT exp table set early so the ~2.7us ACT_TABLE_LOAD is
            # off the attention critical path
            warm = pp.tile([1, 8], F32, tag="warm")
            nc.gpsimd.memset(warm[:], 0.0)
            nc.scalar.activation(warm[:], warm[:], EXP)
            # ones stationary for the PE denominator broadcast (K=1 matmul)
            ones1 = pp.tile([1, 64], BF16, tag="ones1")
            nc.gpsimd.memset(ones1[:], 1.0)

            # ---------- DRAM bounce for AllGather (4 chunks of 512 t) ----------
            cc_in = [dp.tile([DQ, 512], BF16, tag=f"cci{i}", name=f"cci{i}") for i in range(4)]
            cc_out = [dp.tile([4 * DQ, 512], BF16, tag=f"cco{i}", name=f"cco{i}") for i in range(4)]
            # warmup collective: absorbs the DGE start delay (~11us) and the
            # initial cross-core sync skew so the first real AllGather is fast
            cw_in = dp.tile([128, 16], BF16, tag="cwi", name="cwi")
            cw_out = dp.tile([512, 16], BF16, tag="cwo", name="cwo")

            with (
                tc.tile_pool(name="pb", bufs=1) as pb,
                tc.tile_pool(name="pb_ps", bufs=1, space=bass.MemorySpace.PSUM) as bps,
            ):
                wkvq_sb = [
                    pb.tile([128, 2 * DKV + DQ], BF16, tag=f"wkvq{i}", name=f"wkvq{i}")
                    for i in range(16)
                ]
                wk_sb = [t[:, 0:DKV] for t in wkvq_sb]
                wv_sb = [t[:, DKV:2 * DKV] for t in wkvq_sb]
                wq_sb = [t[:, 2 * DKV:2 * DKV + DQ] for t in wkvq_sb]
                ktraw = pb.tile([128, T], BF16, tag="ktraw")

                # ---------- warmups ----------
                junk = pb.tile([128, 512], BF16, tag="junk")
                nc.gpsimd.memset(junk[:], 0.0)
                nc.sync.dma_start(out=cw_in[:], in_=junk[:, 0:16])
                nc.gpsimd.collective_compute(
                    "AllGather",
                    mybir.AluOpType.bypass,
                    replica_groups=[[0, 1, 2, 3], [4, 5, 6, 7]],
                    ins=[cw_in[:].opt()],
                    outs=[cw_out[:].opt()],
                )
                for _ in range(5):
                    jps = bps.tile([128, 1024], F32, tag="qk2", bufs=3, name="jps")
                    nc.tensor.matmul(jps[:, 0:512], junk[:, 0:128], junk[:], start=True, stop=True)

                # ---------- x^T tiles ----------
                # wide per-quarter tiles [128, 16*512]; col group kc holds
                # xT[kc*128:(kc+1)*128, q*512:(q+1)*512].  bufs=3: quarter q+3
                # reuses quarter q's slot once its last consumer retires.
                xtw = {}

                def xt(kc, q):
                    return xtw[q][:, kc * 512:(kc + 1) * 512]

                def emit_xtq_dma(q):
                    t = pb.tile([128, 16 * 512], BF16, tag="xtw", bufs=3, name=f"xtw_{q}")
                    xtw[q] = t
                    # 4 dispatches of 4 kc-tiles each
                    for g in range(4):
                        nc.sync.dma_start(
                            out=t[:, g * 2048:(g + 1) * 2048].rearrange(
                                "p (kc t) -> p kc t", kc=4
                            ),
                            in_=xT_d[g * 512:(g + 1) * 512,
                                     q * 512:(q + 1) * 512].rearrange(
                                "(kc p) t -> p kc t", p=128
                            ),
                        )

                # quarter 0 as 16 separate dispatches (all DMA rings in
                # parallel -- latency-critical), interleaved kc-major with the
                # weights so the K-proj chain's operands arrive consumer-order
                xtw[0] = pb.tile([128, 16 * 512], BF16, tag="xtw", bufs=3, name="xtw_0")
                for kc in range(16):
                    nc.sync.dma_start(
                        out=wkvq_sb[kc][:], in_=wkvq_d[kc * 128:(kc + 1) * 128, :]
                    )
                    nc.sync.dma_start(
                        out=xtw[0][:, kc * 512:(kc + 1) * 512],
                        in_=xT_d[kc * 128:(kc + 1) * 128, 0:512],
                    )
                emit_xtq_dma(1)
                nc.sync.dma_start(out=cosr[:], in_=cosr_d[:])
                nc.sync.dma_start(out=sinr[:], in_=sinr_d[:])
                # xtw q2/q3 and wo stream in during the prologue tail / chunk
                # 0 via deferred emission (the DMA dispatch queue is serial at
                # ~600ns per descriptor: consumer order = dispatch order)

                # ---------- projection building blocks ----------
                def emit_k_quarter(q):
                    ps = bps.tile([128, 1024], F32, tag="qk2", bufs=3, name="kps")
                    for kc in range(16):
                        nc.tensor.matmul(
                            ps[:, 0:512], wk_sb[kc], xt(kc, q),
                            start=(kc == 0), stop=(kc == 15),
                        )
                    nc.vector.tensor_copy(ktraw[:, q * 512:(q + 1) * 512], ps[:, 0:512])

                def emit_v_slice(s):
                    q, qo = s // 4, (s % 4) * 128
                    psv = bps.tile([128, 512], F32, tag="pv", bufs=2, name="vps")
                    for kc in range(16):
                        nc.tensor.matmul(
                            psv[:, 0:128], xt(kc, q)[:, qo:qo + 128], wv_sb[kc],
                            start=(kc == 0), stop=(kc == 15),
                        )
                    nc.vector.tensor_copy(vaug[0][s][:, 0:HD], psv[:, 0:HD])
                    nc.vector.tensor_copy(vaug[1][s][:, 0:HD], psv[:, HD:2 * HD])

                def emit_krope_half(half):
                    # K rope on a [128,1024] half -> duplicated halves of ktd
                    t0 = half * 1024
                    ktr = pb.tile([128, 1024], BF16, tag="ktr", bufs=1)
                    swp = pb.tile([128, 1024], BF16, tag="swpk", bufs=1)
                    for a, b in ((0, 32), (32, 0), (64, 96), (96, 64)):
                        nc.sync.dma_start(out=swp[a:a + 32, :], in_=ktraw[b:b + 32, t0:t0 + 1024])
                    t1k = pb.tile([128, 1024], BF16, tag="t1k", bufs=1)
                    t2k = pb.tile([128, 1024], BF16, tag="t2k", bufs=1)
                    nc.vector.tensor_mul(t1k[:], ktraw[:, t0:t0 + 1024], cosr[:, t0:t0 + 1024])
                    nc.vector.tensor_mul(t2k[:], swp[:], sinr[:, t0:t0 + 1024])
                    nc.vector.tensor_add(ktr[:], t1k[:], t2k[:])
                    nc.sync.dma_start(out=ktd[0][0:64, t0:t0 + 1024], in_=ktr[0:64, :])
                    nc.sync.dma_start(out=ktd[0][64:128, t0:t0 + 1024], in_=ktr[0:64, :])
                    nc.sync.dma_start(out=ktd[1][0:64, t0:t0 + 1024], in_=ktr[64:128, :])
                    nc.sync.dma_start(out=ktd[1][64:128, t0:t0 + 1024], in_=ktr[64:128, :])

                def emit_q_group(q, dq):
                    """Q proj psum tile (dq, quarter q); evac on ACT."""
                    ps = bps.tile([128, 1024], F32, tag="qk2", bufs=3, name="qps")
                    for kc in range(16):
                        nc.tensor.matmul(
                            ps[:, 0:512],
                            wkvq_sb[kc][:, 2 * DKV + dq * 128:2 * DKV + (dq + 1) * 128],
                            xt(kc, q),
                            start=(kc == 0), stop=(kc == 15),
                        )
                    qr = pb.tile([128, 512], BF16, tag="qraw", bufs=3, name="qr")
                    nc.scalar.activation(qr[:], ps[:, 0:512], COPY)
                    return qr

                def emit_qrope_quarter(dq, q, qr):
                    # qt[dq][:, q-quarter] = qr*cosr + swap32(qr)*sinr
                    t0 = q * 512
                    swp = pb.tile([128, 512], BF16, tag="swpq", bufs=2)
                    for a, b in ((0, 32), (32, 0), (64, 96), (96, 64)):
                        nc.sync.dma_start(out=swp[a:a + 32, :], in_=qr[b:b + 32, :])
                    t1 = pb.tile([128, 512], BF16, tag="t1q", bufs=2)
                    t2 = pb.tile([128, 512], BF16, tag="t2q", bufs=2)
                    nc.vector.tensor_mul(t1[:], qr[:], cosr[:, t0:t0 + 512])
                    nc.vector.tensor_mul(t2[:], swp[:], sinr[:, t0:t0 + 512])
                    nc.vector.tensor_add(qt[dq][:, t0:t0 + 512], t1[:], t2[:])

                # ---------- prologue ----------
                # Q quarter 0 goes early so its xtw slot frees for quarter 3,
                # letting K finish (and ktd fully write) before the trailing
                # V block -- the first attention QK then starts stall-free.
                emit_k_quarter(0)
                emit_k_quarter(1)
                for s in range(4):
                    emit_v_slice(s)
                emit_krope_half(0)
                for dq in range(4):
                    qr = emit_q_group(0, dq)
                    emit_qrope_quarter(dq, 0, qr)
                emit_xtq_dma(2)
                emit_xtq_dma(3)
                for s in range(4, 8):
                    emit_v_slice(s)
                emit_k_quarter(2)
                emit_k_quarter(3)
                emit_krope_half(1)
                for s in range(8, 16):
                    emit_v_slice(s)

                # ---------- attention pair blocks ----------
                def new_ctx(chunk, pair):
                    return {
                        "chunk": chunk, "pair": pair,
                        "ta": chunk * 512, "kv": pair // 2,
                        "pv_a": bps.tile([128, 512], F32, tag="pv", bufs=2, name="pv_a"),
                        "pv_b": bps.tile([128, 512], F32, tag="pv", bufs=2, name="pv_b"),
                        "qks": {}, "ess": {},
                    }

                def emit_qk(ctx, s):
                    qk = bps.tile([128, 1024], F32, tag="qk2", bufs=3)
                    ctx["qks"][s] = qk
                    kv, pair, ta = ctx["kv"], ctx["pair"], ctx["ta"]
                    # row-packed pair: head A on rows 0-63 -> tile (0,0),
                    # head B on rows 64-127 -> tile (64,0): concurrent MMs
                    nc.tensor.matmul(
                        qk[:, 0:512],
                        ktd[kv][0:64, s * 128:(s + 1) * 128],
                        qt[pair][0:64, ta:ta + 512],
                        start=True, stop=True,
                    )
                    nc.tensor.matmul(
                        qk[:, 512:1024],
                        ktd[kv][64:128, s * 128:(s + 1) * 128],
                        qt[pair][64:128, ta:ta + 512],
                        start=True, stop=True,
                    )

                def emit_exp(ctx, s):
                    # full-tile exp alternating engines per s (one dispatch
                    # bubble per 1024 cols); bufs=4 (even) so each pool slot
                    # is always rewritten by the SAME engine
                    qk = ctx["qks"][s]
                    es = pb.tile([128, 1024], BF16, tag="es", bufs=4)
                    ctx["ess"][s] = es
                    if s % 2 == 0:
                        nc.scalar.activation(es[:], qk[:], EXP)
                    else:
                        nc.vector.tensor_scalar(
                            es[:].bitcast(I16), qk[:],
                            EXP_A, EXP_B, MULT, ADD,
                        )

                def emit_pv(ctx, s):
                    es = ctx["ess"].pop(s)
                    ctx["qks"].pop(s)
                    kv = ctx["kv"]
                    nc.tensor.matmul(
                        ctx["pv_a"][:], vaug[kv][s][:], es[:, 0:512],
                        start=(s == 0), stop=(s == 15),
                        skip_group_check=True,
                    )
                    nc.tensor.matmul(
                        ctx["pv_b"][:], vaug[kv][s][:], es[:, 512:1024],
                        start=(s == 0), stop=(s == 15),
                        skip_group_check=True,
                    )

                def emit_prologue(ctx):
                    # 2-deep lookahead: QK runs two iterations ahead of PV so
                    # the exp latency stays off the PE critical path; both
                    # first exps precede the previous pair's tail in engine
                    # queue order
                    emit_qk(ctx, 0)
                    emit_qk(ctx, 1)
                    emit_exp(ctx, 0)
                    emit_exp(ctx, 1)

                def emit_body(ctx):
                    for s in range(16):
                        if s + 2 < 16:
                            emit_qk(ctx, s + 2)
                            emit_exp(ctx, s + 2)
                        emit_pv(ctx, s)

                def emit_tail(ctx, fast_chain=False):
                    chunk, pair = ctx["chunk"], ctx["pair"]
                    pv_a, pv_b = ctx["pv_a"], ctx["pv_b"]
                    # evacuate pv (frees the PSUM banks for the next pair:
                    # ACT takes head A, DVE takes head B, in parallel)
                    pvs_a = pb.tile([HD + 1, 512], F32, tag="pvsa", bufs=1)
                    pvs_b = pb.tile([HD + 1, 512], F32, tag="pvsb", bufs=1)
                    nc.scalar.activation(pvs_a[:], pv_a[0:HD + 1, :], COPY)
                    nc.vector.tensor_copy(pvs_b[:], pv_b[0:HD + 1, :])
                    # denominator -> PE broadcast -> normalize -> cc_in.
                    # The broadcast is a K=1 ones-stationary matmul into the
                    # just-freed pv psum slots: partition_broadcast would ride
                    # the gpsimd queue, which collective_compute blocks for its
                    # whole ~25us duration, flakily stalling the tails.
                    denr = pb.tile([1, 1024], BF16, tag="denr", bufs=2)
                    nc.scalar.activation(denr[0:1, 0:512], pv_a[64:65, :], COPY)
                    nc.scalar.activation(denr[0:1, 512:1024], pv_b[64:65, :], COPY)
                    bc_a = bps.tile([128, 512], F32, tag="pv", bufs=2, name="bc_a")
                    bc_b = bps.tile([128, 512], F32, tag="pv", bufs=2, name="bc_b")
                    nc.tensor.matmul(bc_a[0:64, :], ones1[0:1, :], denr[0:1, 0:512],
                                     start=True, stop=True)
                    nc.tensor.matmul(bc_b[0:64, :], ones1[0:1, :], denr[0:1, 512:1024],
                                     start=True, stop=True)
                    tma = pb.tile([64, 512], BF16, tag="tma", bufs=2)
                    tmb = pb.tile([64, 512], BF16, tag="tmb", bufs=2)
                    rep = pb.tile([64, 1024], F32, tag="rep", bufs=1)
                    nc.vector.reciprocal_approx_fast(out=rep[:, 0:512], in_=bc_a[0:64, :])
                    nc.vector.reciprocal_approx_fast(out=rep[:, 512:1024], in_=bc_b[0:64, :])
                    nc.vector.tensor_mul(tma[:], pvs_a[0:64, :], rep[:, 0:512])
                    nc.vector.tensor_mul(tmb[:], pvs_b[0:64, :], rep[:, 512:1024])
                    r0 = pair * 128
                    nc.sync.dma_start(out=cc_in[chunk][r0:r0 + 64, :], in_=tma[:])
                    nc.sync.dma_start(out=cc_in[chunk][r0 + 64:r0 + 128, :], in_=tmb[:])

                def do_ag_chunk(chunk):
                    nc.gpsimd.collective_compute(
                        "AllGather",
                        mybir.AluOpType.bypass,
                        replica_groups=[[0, 1, 2, 3], [4, 5, 6, 7]],
                        ins=[cc_in[chunk][:].opt()],
                        outs=[cc_out[chunk][:].opt()],
                    )

                def emit_ag_load(chunk):
                    # one wide dispatch: [2048, 512] DRAM -> [128, 16, 512] SBUF
                    # (bass sprays large single descriptors across DMA rings).
                    # Dispatched from the GPSIMD queue: the sync queue runs
                    # ~25us ahead of the PE and would head-of-line block on the
                    # AllGather-done semaphore, stalling the swp/cc dispatches
                    # behind it; the gpsimd queue is paced by the per-pair
                    # partition broadcasts, so the AG is always done by then.
                    nc.gpsimd.dma_start(
                        out=agw[:].rearrange("p (m t) -> p m t", m=16),
                        in_=cc_out[chunk][:].rearrange("(m p) t -> p m t", p=128),
                    )

                def emit_ag_load_tt(chunk, tt):
                    # per-tt column gather: [128, 16, 128] -- lets the final wo
                    # groups chase the AllGather instead of one bulk reload
                    nc.sync.dma_start(
                        out=agw[:].rearrange("p (m tt f) -> p m tt f", m=16, tt=4)[
                            :, :, tt:tt + 1, :],
                        in_=cc_out[chunk][:].rearrange("(m p) (tt f) -> p m tt f",
                                                       p=128, tt=4)[:, :, tt:tt + 1, :],
                    )

                def emit_wo_group(chunk, tt, osb_w, evac_dve=False):
                    """one [128, 512] psum tile of out[:, chunk cols]"""
                    pso = bps.tile([128, 1024], F32, tag="qk2", bufs=3, name="pso")
                    for m in range(16):
                        nc.tensor.matmul(
                            pso[:, 0:512],
                            agw[:, m * 512 + tt * 128:m * 512 + (tt + 1) * 128],
                            wo_w[:, m * 512:(m + 1) * 512],
                            start=(m == 0), stop=(m == 15),
                        )
                    if evac_dve:
                        nc.vector.tensor_copy(osb_w[:, tt * 512:(tt + 1) * 512], pso[:, 0:512])
                    else:
                        nc.scalar.activation(osb_w[:, tt * 512:(tt + 1) * 512], pso[:, 0:512], COPY)

                def emit_out_store(chunk, osb_w):
                    tb = chunk * 512
                    nc.sync.dma_start(
                        out=out_d[tb:tb + 512, :].rearrange("(tt p) c -> p tt c", p=128),
                        in_=osb_w[:].rearrange("p (tt c) -> p tt c", tt=4),
                    )

                def emit_out_store_tt(chunk, tt, osb_w):
                    tb = chunk * 512
                    nc.sync.dma_start(
                        out=out_d[tb + tt * 128:tb + (tt + 1) * 128, :],
                        in_=osb_w[:, tt * 512:(tt + 1) * 512],
                    )

                # ---------- schedule ----------
                # fillers[(chunk, pair)] -> emitted between the previous
                # pair's tail and this pair's body.
                # Q-proj fillers: the matmul group runs in pair p's slot;
                # its RoPE (DVE) runs in pair p+1's slot so it never delays
                # the exp stream right after the projection.
                qr_stash = {}

                def q_mm_filler(q, dq):
                    def f():
                        qr_stash[(q, dq)] = emit_q_group(q, dq)
                    return f

                def q_rope_filler(q, dq):
                    def f():
                        emit_qrope_quarter(dq, q, qr_stash.pop((q, dq)))
                    return f

                def wo_dma_filler():
                    def f():
                        for hh in range(2):
                            nc.sync.dma_start(
                                out=wo_w[:, hh * 4096:(hh + 1) * 4096].rearrange(
                                    "p (m c) -> p m c", m=8
                                ),
                                in_=wo_d[hh * 1024:(hh + 1) * 1024, :].rearrange(
                                    "(m p) c -> p m c", p=128
                                ),
                            )
                    return f

                fillers = {}
                fillers[(0, 1)] = [wo_dma_filler()]
                for c in (0, 1, 2):
                    q = c + 1
                    fillers.setdefault((c, 0), []).append(q_mm_filler(q, 0))
                    fillers.setdefault((c, 1), []).append(q_rope_filler(q, 0))
                    fillers[(c, 1)].append(q_mm_filler(q, 1))
                    fillers[(c, 2)] = [q_rope_filler(q, 1), q_mm_filler(q, 2)]
                    fillers[(c, 3)] = [q_rope_filler(q, 2), q_mm_filler(q, 3)]

                pending = {}
                prev = None
                deferred_store = None
                for chunk in range(4):
                    for pair in range(4):
                        ctx = pending.pop((chunk, pair), None)
                        if ctx is None:
                            ctx = new_ctx(chunk, pair)
                            emit_prologue(ctx)
                        if prev is not None:
                            emit_tail(prev)
                            prev = None
                        # the previous boundary's output store dispatches here,
                        # one pair later, so the sync queue never blocks on the
                        # wo psum evacuations still in flight
                        if pair == 1 and deferred_store is not None:
                            emit_out_store(*deferred_store)
                            deferred_store = None
                        # gathered-chunk unload: late enough that the AllGather
                        # is done (no DMA-queue head blocking), early enough
                        # that the boundary wo block never waits
                        if chunk >= 1 and pair == 3:
                            emit_ag_load(chunk - 1)
                        for f in fillers.get((chunk, pair), []):
                            f()
                        emit_body(ctx)
                        prev = ctx
                    if chunk < 3:
                        nctx = new_ctx(chunk + 1, 0)
                        emit_prologue(nctx)
                        pending[(chunk + 1, 0)] = nctx
                    emit_tail(prev, fast_chain=(chunk == 3))
                    prev = None
                    do_ag_chunk(chunk)
                    if chunk < 3:
                        emit_qrope_quarter(3, chunk + 1, qr_stash.pop((chunk + 1, 3)))
                    # chunk-boundary wo block: all four groups of chunk-1's
                    # AllGather (a full chunk of slack) run here as PE filler
                    # while this chunk's AllGather transfers
                    if chunk >= 1:
                        osb_w = pb.tile([128, 4 * 512], F32, tag="osb", bufs=1,
                                        name=f"osb{chunk - 1}")
                        for tt in range(4):
                            emit_wo_group(chunk - 1, tt, osb_w, evac_dve=(tt % 2 == 1))
                        if chunk < 3:
                            deferred_store = (chunk - 1, osb_w)
                        else:
                            emit_out_store(chunk - 1, osb_w)
                # final chunk: per-tt column unloads let each wo group start as
                # soon as its own slice of the gather has been pulled back in;
                # the per-tt WAR on the chunk-2 wo reads also releases early
                for tt in range(4):
                    emit_ag_load_tt(3, tt)
                osb_w = pb.tile([128, 4 * 512], F32, tag="osb", bufs=1, name="osb3")
                for tt in range(4):
                    emit_wo_group(3, tt, osb_w, evac_dve=(tt % 2 == 1))
                    emit_out_store_tt(3, tt, osb_w)

    return nc


# ---------------------------------------------------------------------------
# Host side
# ---------------------------------------------------------------------------

_CACHE = {}


def _rope_tables():
    i = np.arange(32)
    freqs = 1.0 / (THETA ** (2.0 * i / HD))          # [32]
    ang = np.arange(T, dtype=np.float64)[:, None] * freqs[None, :]  # [T, 32]
    cos = np.cos(ang)
    sin = np.sin(ang)
    p = np.arange(128)
    fi = p % 32
    sign = np.where(p % 64 < 32, -1.0, 1.0)
    cosr = cos[:, fi].T                               # [128, T]
    sinr = (sin[:, fi] * sign[None, :]).T             # [128, T]
    return cosr.astype(np.float32), sinr.astype(np.float32)


def _colperm(n_heads):
    """rotate-half permutation: per 64-col head block, evens then odds"""
    blk = np.concatenate([np.arange(0, HD, 2), np.arange(1, HD, 2)])
    return np.concatenate([h * HD + blk for h in range(n_heads)])


def _prep_in_maps(x, wq, wk, wv, wo):
    cosr, sinr = _rope_tables()
    qperm = _colperm(32)
    kperm = _colperm(8)
    wq_p = (wq.astype(np.float64) / 8.0)[:, qperm]    # fold 1/sqrt(hd)
    wk_p = wk[:, kperm]
    in_maps = []
    for c in range(N_CORES):
        b, g = divmod(c, 4)
        in_maps.append({
            "xT": np.ascontiguousarray(x[b].T).astype(NPBF16),
            "wkvq": np.concatenate(
                [wk_p[:, g * DKV:(g + 1) * DKV], wv[:, g * DKV:(g + 1) * DKV],
                 wq_p[:, g * DQ:(g + 1) * DQ]],
                axis=1,
            ).astype(NPBF16),
            "wo": wo[:, g * DQ:(g + 1) * DQ].astype(NPBF16),
            "cosr": cosr.astype(NPBF16),
            "sinr": sinr.astype(NPBF16),
        })
    return in_maps


def get_nc():
    if "nc" not in _CACHE:
        nc = build_nc()
        if not nc.is_finalized():
            nc.finalize()
        _CACHE["nc"] = nc
    return _CACHE["nc"]


def run_on_hw(in_maps, trace=False):
    nc = get_nc()
    return run_bass_kernel_spmd(nc, in_maps, core_ids=list(range(N_CORES)), trace=trace)


def _assemble(results):
    out = np.zeros((2, T, C), dtype=np.float32)
    for c in range(N_CORES):
        b, g = divmod(c, 4)
        out[b][:, g * DQ:(g + 1) * DQ] = np.asarray(results[c]["out"], dtype=np.float32)
    return out


def kernel(x, wq, wk, wv, wo):
    in_maps = _prep_in_maps(
        np.asarray(x, np.float32), np.asarray(wq, np.float32),
        np.asarray(wk, np.float32), np.asarray(wv, np.float32),
        np.asarray(wo, np.float32),
    )
    res = run_on_hw(in_maps, trace=False)
    return _assemble(res.results)
